# revision 1
# baseline (speedup 1.0000x reference)
"""Trainium2 Bass kernel for nn_Attention_82660940579436.

Computation (see reference):
    q     = mean_s(hidden @ Wq.T + bq)            [B, H]
    key   = tanh(hidden @ Wk.T + bk)              [S, B, H]
    score = einsum('bsh,bh->bs', key, q) + mask   [B, S]
    out   = softmax(score) @ key                  [B, H]

Sharding: data-parallel over batch. B=32 over 8 cores -> 4 batches/core.
Each core streams its 32 MiB hidden slice once, keeps key resident in SBUF
as bf16, then does a second SBUF-only pass for the softmax-weighted sum.

Device algorithm per core (4 local batches, tokens = (s, b) pairs):
  Phase A (per 128-token tile: 32 s-positions x 4 batches):
    - DMA hidden tile [128 tok, 512] fp32
    - PE transpose -> hT [512 j, 128 tok] (4x 128x128 via identity)
    - DVE copy hT PSUM->SBUF; DVE strided reduce accumulates sum_s(h) per (j,b)
    - PE: z = bk (rank-1 matmul) + hT.T @ WkT (4 matmuls, fp32) -> PSUM
    - ACT: key = tanh(z) -> resident SBUF bf16
  q = (sum_s h / S) @ WqT + bq  (tiny matmuls); qrep[p,:] = q[p%4,:] via PE
  Phase B (per tile):
    - DVE mul + reduce: score[p] = sum_i key[p,i]*qrep[p,i]
      (tensor_tensor_reduce would fuse this, but custom DVE ops fault at
      runtime under the axon compile path -- ucode tables are not shipped)
    - ACT: e = exp(score + mask_bias)  (mask as per-partition bias)
    - DVE: e_ind[p,g] = e[p] * (p%4==g)
    - PE: numer[4,512] += e_ind.T @ key ; den[4,1] += e_ind.T @ ones
  out = numer / den  -> DMA out [4, 512]

All big-matmul operands are bf16: TRN2's PE runs fp32 matmuls at 1/4 rate
(two half-speed passes), bf16 at 1 column/cycle. The hidden input is cast
fp32->bf16 during the SWDGE DMA load (free). The q path stays fp32.
Cost-model timeline (concourse InstructionCostModel): ~330 us/core.

exp() needs no max-subtraction: scores are O(1) by construction, masked
positions get -60 bias -> exp underflows to ~1e-27 (reference's -10000
mask likewise produces exact zeros after its own softmax).

All constants ship in two packed tensors (one fp32, one bf16) loaded by a
single DMA each, and two dummy PE ops observe those DMA lanes up front:
walrus only allows ONE sync-wait on a Matmult (S3_LW), so every real
matmul must have at most one not-yet-observed dependency.
"""

import sys
from contextlib import ExitStack

import numpy as np

if "/opt/trn_rl_repo" not in sys.path:
    sys.path.insert(0, "/opt/trn_rl_repo")

import ml_dtypes  # noqa: E402

import concourse.bacc as bacc  # noqa: E402
import concourse.bass as bass  # noqa: E402
import concourse.mybir as mybir  # noqa: E402
import concourse.tile as tile  # noqa: E402
from concourse.bass_utils import run_bass_kernel_spmd  # noqa: E402

S, B, H = 4096, 32, 512
NCORES = 8
BPC = B // NCORES  # 4 batches per core
NT = 128  # tiles per core
SS = S // NT  # 32 s-positions per tile
TOK = SS * BPC  # 128 tokens per tile
HC = H // 128  # 4 chunks of the H (j / i) dims
MASK_NEG = -60.0
F32 = mybir.dt.float32
BF16 = mybir.dt.bfloat16
AF = mybir.ActivationFunctionType
ALU = mybir.AluOpType
BF16NP = ml_dtypes.bfloat16

# fp32 const pack layout (offsets in fp32 elements, [128, PACKF] tensor)
OFF_WQ = 0  # [128, 2048] WqT chunks
OFF_MASK = 2048  # [128, 128] mask bias (0 / MASK_NEG), col=tile
OFF_BQ = 2176  # [4, 512] bq rows
OFF_IND4T = 2688  # [4, 128] indicator transposed
OFF_ZERO = 2816  # [128, 1] zeros (tanh bias)
PACKF = 2824
# bf16 const pack layout ([128, PACKB]) — matmul operands live here:
# fp32 matmuls run at 1/4 rate on TRN2, bf16 at full rate.
OFFB_WK = 0  # [128, 2048] WkT chunks
OFFB_ID = 2048  # [128, 128] identity
OFFB_BK = 2176  # [1, 512] bk on partition 0
OFFB_ONESROW = 2688  # [1, 128] ones on partition 0
OFFB_IND4 = 2816  # [128, 4] indicator
OFFB_ONES = 2820  # [128, 1] ones
PACKB = 2824

# tuning knobs (read at build time)
KNOBS = {
    "h_bufs": 8,
    "hT_bufs": 3,
    "hTps_bufs": 2,
    "keyps_bufs": 2,
    "small_bufs": 3,
    "phase_a_only": False,
    "no_transpose": False,  # debug: skip transposes (wrong results)
    "no_phase_b_mm": False,
}


def _build_kernel_body(tc, aps):
    nc = tc.nc
    x, packf, packb, y = aps["x"], aps["packf"], aps["packb"], aps["y"]

    with ExitStack() as ctx:
        consts = ctx.enter_context(tc.tile_pool(name="consts", bufs=1))
        ph = ctx.enter_context(tc.tile_pool(name="h", bufs=KNOBS["h_bufs"]))
        phT = ctx.enter_context(tc.tile_pool(name="hT", bufs=KNOBS["hT_bufs"]))
        pkeys = ctx.enter_context(tc.tile_pool(name="keys", bufs=NT))
        psmall = ctx.enter_context(tc.tile_pool(name="small", bufs=KNOBS["small_bufs"]))
        pacc = ctx.enter_context(tc.tile_pool(name="acc", bufs=1))
        pps_hT = ctx.enter_context(tc.tile_pool(name="ps_hT", bufs=KNOBS["hTps_bufs"], space="PSUM"))
        pps_key = ctx.enter_context(tc.tile_pool(name="ps_key", bufs=KNOBS["keyps_bufs"], space="PSUM"))
        pps_acc = ctx.enter_context(tc.tile_pool(name="ps_acc", bufs=1, space="PSUM"))
        pps_sm = ctx.enter_context(tc.tile_pool(name="ps_sm", bufs=1, space="PSUM"))

        # ---- constants: one DMA per pack ----
        cf = consts.tile([128, PACKF], F32)
        nc.sync.dma_start(cf, packf)
        cb = consts.tile([128, PACKB], BF16)
        nc.sync.dma_start(cb, packb)

        def wk_sb(c):
            return cb[:, OFFB_WK + c * 512 : OFFB_WK + (c + 1) * 512]

        def wq_sb(c):
            return cf[:, OFF_WQ + c * 512 : OFF_WQ + (c + 1) * 512]

        id_sb = cb[:, OFFB_ID : OFFB_ID + 128]
        maskb_sb = cf[:, OFF_MASK : OFF_MASK + NT]
        bk_sb = cb[0:1, OFFB_BK : OFFB_BK + H]
        bq_sb = cf[0:BPC, OFF_BQ : OFF_BQ + H]
        ones_row_sb = cb[0:1, OFFB_ONESROW : OFFB_ONESROW + 128]
        ind4T_sb = cf[0:BPC, OFF_IND4T : OFF_IND4T + 128]
        zero_sb = cf[:, OFF_ZERO : OFF_ZERO + 1]
        ind4_sb = cb[:, OFFB_IND4 : OFFB_IND4 + BPC]
        ones1_sb = cb[:, OFFB_ONES : OFFB_ONES + 1]

        # Dummy PE ops: observe each const-pack DMA lane once, so no real
        # matmul ever needs two sync-waits (walrus S3_LW limit is one).
        scr = pps_sm.tile([128, H], BF16, tag="smb")
        nc.tensor.transpose(scr[:, :128], id_sb, id_sb)
        scr2 = pps_sm.tile([128, H], F32, tag="sm")
        nc.tensor.matmul(scr2[:128, :128], ind4T_sb, wq_sb(0)[0:BPC, 0:128],
                         start=True, stop=True)

        macc = pacc.tile([128, HC * BPC], F32)  # sum_s h, laid out (j_local, (c, g))
        nc.vector.memset(macc, 0.0)

        # ---- Phase A ----
        keys = []
        for t in range(NT):
            h_t = ph.tile([TOK, H], BF16, tag="h")
            # SWDGE (gpsimd): casts fp32->bf16 during the DMA (free), and its
            # ucode path tolerates the multi-sync-waits this load needs.
            nc.gpsimd.dma_start(h_t, x[t])

            hT_ps = pps_hT.tile([128, H], BF16, tag="hT")
            for c in range(HC):
                nc.tensor.transpose(
                    hT_ps[:, c * 128 : (c + 1) * 128],
                    h_t[:, c * 128 : (c + 1) * 128],
                    id_sb,
                )
            hT_sb = phT.tile([128, H], BF16, tag="hT_sb")
            nc.vector.tensor_copy(hT_sb, hT_ps)

            red = psmall.tile([128, HC * BPC], F32, tag="red")
            nc.vector.tensor_reduce(
                red,
                hT_sb.rearrange("p (c s g) -> p c g s", c=HC, s=SS, g=BPC),
                axis=mybir.AxisListType.X,
                op=ALU.add,
            )
            nc.vector.tensor_add(macc, macc, red)

            key_ps = pps_key.tile([TOK, H], F32, tag="key")
            nc.tensor.matmul(key_ps, ones_row_sb, bk_sb, start=True, stop=False)
            for c in range(HC):
                nc.tensor.matmul(
                    key_ps,
                    hT_sb[:, c * 128 : (c + 1) * 128],
                    wk_sb(c),
                    start=False,
                    stop=(c == HC - 1),
                )
            key_t = pkeys.tile([TOK, H], BF16, tag="key")
            nc.scalar.activation(key_t, key_ps, AF.Tanh, bias=zero_sb)
            keys.append(key_t)

        # ---- q = (sum_s h / S) @ WqT + bq ; qrep[p] = q[p%4] ----
        q_ps = pps_sm.tile([BPC, H], F32, tag="sm")
        for c in range(HC):
            nc.tensor.matmul(
                q_ps,
                macc[:, c * BPC : (c + 1) * BPC],
                wq_sb(c),
                start=(c == 0),
                stop=(c == HC - 1),
            )
        q_sb = pacc.tile([BPC, H], F32)
        nc.scalar.mul(q_sb, q_ps, 1.0 / S)
        nc.vector.tensor_add(q_sb, q_sb, bq_sb)
        qrep_ps = pps_sm.tile([128, H], F32, tag="sm")
        nc.tensor.matmul(qrep_ps, ind4T_sb, q_sb, start=True, stop=True)
        qrep_sb = pacc.tile([128, H], BF16)
        nc.scalar.copy(qrep_sb, qrep_ps)

        # ---- Phase B ----
        numer_ps = pps_acc.tile([BPC, H], F32, tag="numer")
        den_ps = pps_acc.tile([BPC, 1], F32, tag="den")
        for t in range(NT):
            # Score products split 40/60 between DVE and the otherwise-idle
            # GPSIMD engine; the X-axis reduce is DVE-only. (TimelineSim:
            # 330 -> 306 us; all-DVE and all-GPSIMD are both worse.)
            if t % 5 < 2:
                prod = psmall.tile([TOK, H], BF16, tag="prod")
                nc.vector.tensor_mul(prod, keys[t], qrep_sb)
            else:
                prod = psmall.tile([TOK, H], BF16, tag="prodg")
                nc.gpsimd.tensor_mul(prod, keys[t], qrep_sb)
            sc_t = psmall.tile([TOK, 1], F32, tag="sc")
            # The row-sum reduce also splits across engines: tensor_reduce on
            # DVE for half the tiles, ACT's activation(Copy, accum_out=) for
            # the other half (ACT is mostly idle in phase B). 306 -> 290 us.
            if t % 2 == 0:
                nc.vector.tensor_reduce(
                    sc_t, prod, axis=mybir.AxisListType.X, op=ALU.add
                )
            else:
                pc = psmall.tile([TOK, H], BF16, tag="pc")
                nc.scalar.activation(pc, prod, AF.Copy, accum_out=sc_t)
            e_t = psmall.tile([TOK, 1], F32, tag="e")
            nc.scalar.activation(e_t, sc_t, AF.Exp, bias=maskb_sb[:, t : t + 1])
            ei_t = psmall.tile([TOK, BPC], BF16, tag="ei")
            nc.vector.tensor_scalar_mul(ei_t, ind4_sb, e_t)
            nc.tensor.matmul(
                numer_ps, ei_t, keys[t], start=(t == 0), stop=(t == NT - 1)
            )
            nc.tensor.matmul(
                den_ps, ei_t, ones1_sb, start=(t == 0), stop=(t == NT - 1)
            )

        # ---- out = numer / den ----
        rcp = pacc.tile([BPC, 1], F32)
        nc.vector.reciprocal(rcp, den_ps)
        out_sb = pacc.tile([BPC, H], F32)
        nc.vector.tensor_scalar_mul(out_sb, numer_ps, rcp)
        nc.sync.dma_start(y, out_sb)


_CACHE = {}


def _fix_dma_waits(nc):
    """walrus's DMA_DIRECT2D lowering only has ONE sync-wait slot, but Tile
    gives each hidden-tile load two waits: (a) WAR, engine sem, readers of the
    recycled buffer; (b) WAW, DMA-lane sem, the load that wrote this buffer 8
    tiles ago.  All these loads sit on the single SWDGE queue (qPoolDynamic):
    descriptor generation is program-ordered and each SDMA engine drains its
    ring FIFO, and a given SBUF byte always belongs to the same engine, so
    same-buffer writes from this queue cannot reorder -- the WAW wait is
    hardware-redundant.  Drop it; keep the WAR wait.

    Also sanity-check the remaining wait counts against walrus's empirical
    limits (DMACopy: 1, everything else: 2, Drain exempt)."""
    for b in nc.m.functions[0].blocks:
        for i in b.instructions:
            si = i.sync_info
            if si is None:
                continue
            waits = list(si.on_wait)
            if (
                type(i).__name__ == "InstDMACopy"
                and getattr(i, "queue", "") == "qPoolDynamic"
                and len(waits) == 2
            ):
                lane = [w for w in waits if w.ant_name.startswith("DMASW")]
                eng = [w for w in waits if not w.ant_name.startswith("DMA")]
                if len(lane) == 1 and len(eng) == 1:
                    out0 = i.outs[0]
                    name = getattr(getattr(out0, "bass_ap", None), "tensor", None)
                    name = getattr(name, "name", "")
                    if name.startswith("h_t"):
                        si.on_wait = eng
                        continue
            if type(i).__name__ in ("InstDrain", "InstEventSemaphore"):
                continue
            limit = 1 if type(i).__name__ == "InstDMACopy" else 2
            if len(waits) > limit:
                raise RuntimeError(
                    f"{i.name} {type(i).__name__} has {len(waits)} waits "
                    f"(> {limit}): {[(w.ant_name, w.wait_value) for w in waits]}"
                )


def _get_program():
    if "nc" in _CACHE:
        return _CACHE["nc"], _CACHE["aps"]
    nc = bacc.Bacc(None, target_bir_lowering=False, debug=False)
    aps = {
        "x": nc.dram_tensor("x", [NT, TOK, H], F32, kind="ExternalInput").ap(),
        "packf": nc.dram_tensor("packf", [128, PACKF], F32, kind="ExternalInput").ap(),
        "packb": nc.dram_tensor("packb", [128, PACKB], BF16, kind="ExternalInput").ap(),
        "y": nc.dram_tensor("y", [BPC, H], F32, kind="ExternalOutput").ap(),
    }
    with tile.TileContext(nc) as tc:
        _build_kernel_body(tc, aps)
    nc.finalize()  # Bacc.compile: wait legalization (EVSEM splits), LDW moves
    _CACHE["nc"] = nc
    _CACHE["aps"] = aps
    return nc, aps


def _make_in_maps(hidden_states, Wq, bq, Wk, bk, lengths):
    hidden = np.asarray(hidden_states, dtype=np.float32)
    Wq = np.asarray(Wq, dtype=np.float32)
    Wk = np.asarray(Wk, dtype=np.float32)
    bqv = np.asarray(bq, dtype=np.float32)
    bkv = np.asarray(bk, dtype=np.float32)
    lens = np.asarray(lengths).astype(np.int64)

    p = np.arange(128)
    packb = np.zeros((128, PACKB), dtype=BF16NP)
    packb[:, OFFB_WK : OFFB_WK + 2048] = (
        np.ascontiguousarray(Wk.T)
        .reshape(HC, 128, H)
        .transpose(1, 0, 2)
        .reshape(128, 2048)
        .astype(BF16NP)
    )
    packb[:, OFFB_ID : OFFB_ID + 128] = np.eye(128, dtype=BF16NP)
    packb[0, OFFB_BK : OFFB_BK + H] = bkv.astype(BF16NP)
    packb[0, OFFB_ONESROW : OFFB_ONESROW + 128] = BF16NP(1.0)
    packb[:, OFFB_IND4 : OFFB_IND4 + BPC] = (
        p[:, None] % BPC == np.arange(BPC)[None, :]
    ).astype(BF16NP)
    packb[:, OFFB_ONES] = BF16NP(1.0)

    base_packf = np.zeros((128, PACKF), dtype=np.float32)
    base_packf[:, OFF_WQ : OFF_WQ + 2048] = (
        np.ascontiguousarray(Wq.T).reshape(HC, 128, H).transpose(1, 0, 2).reshape(128, 2048)
    )
    base_packf[0:BPC, OFF_BQ : OFF_BQ + H] = bqv[None, :]
    base_packf[0:BPC, OFF_IND4T : OFF_IND4T + 128] = (
        p[None, :] % BPC == np.arange(BPC)[:, None]
    ).astype(np.float32)

    in_maps = []
    s_of_p = p // BPC
    t_idx = np.arange(NT)
    for c in range(NCORES):
        xc = np.ascontiguousarray(hidden[:, c * BPC : (c + 1) * BPC, :]).reshape(
            NT, TOK, H
        )
        packf = base_packf.copy()
        b_of_p = c * BPC + (p % BPC)
        s_full = SS * t_idx[None, :] + s_of_p[:, None]  # [128, NT]
        valid = s_full < lens[b_of_p][:, None]
        packf[:, OFF_MASK : OFF_MASK + NT] = np.where(valid, 0.0, MASK_NEG)
        in_maps.append({"x": xc, "packf": packf, "packb": packb})
    return in_maps


def run(hidden_states, Wq, bq, Wk, bk, lengths, trace=False):
    """Run on 8 cores; returns (output [B, H] fp32, BassKernelResults)."""
    nc, _ = _get_program()
    in_maps = _make_in_maps(hidden_states, Wq, bq, Wk, bk, lengths)
    res = run_bass_kernel_spmd(
        nc, in_maps, core_ids=list(range(NCORES)), trace=trace
    )
    out = np.concatenate([np.asarray(r["y"]) for r in res.results], axis=0)
    return out.astype(np.float32), res


def kernel(hidden_states, Wq, bq, Wk, bk, lengths):
    out, _ = run(hidden_states, Wq, bq, Wk, bk, lengths)
    return out


# ---------------------------------------------------------------------------
# Benchmarking helpers (not used by the grader's kernel() path)
# ---------------------------------------------------------------------------


def _make_sharded_callable(nc, in_maps):
    """Replicate run_bass_via_pjrt's multi-core path, but return a reusable
    jitted callable + device-resident inputs so repeat timing excludes
    host->device transfer of the big operands."""
    import jax
    import concourse.mybir as mybir_
    from jax.experimental.shard_map import shard_map
    from jax.sharding import Mesh, NamedSharding, PartitionSpec

    from concourse import bass2jax

    bass2jax.install_neuronx_cc_hook()
    n_cores = len(in_maps)
    partition_name = (
        nc.partition_id_tensor.name if nc.partition_id_tensor else None
    )
    in_names, out_names, out_avals, zero_outs = [], [], [], []
    for alloc in nc.m.functions[0].allocations:
        if not isinstance(mybir_.MemoryLocationSet, type) or not isinstance(
            alloc, mybir_.MemoryLocationSet
        ):
            continue
        if not alloc.memorylocations:
            continue
        name = alloc.memorylocations[0].name
        if alloc.kind == "ExternalInput":
            if name != partition_name:
                in_names.append(name)
        elif alloc.kind == "ExternalOutput":
            shape = tuple(alloc.tensor_shape)
            dtype = mybir_.dt.np(alloc.dtype)
            out_names.append(name)
            out_avals.append(jax.core.ShapedArray(shape, dtype))
            zero_outs.append(np.zeros(shape, dtype))
    n_params = len(in_names)
    all_names = in_names + out_names
    if partition_name is not None:
        all_names = all_names + [partition_name]

    def _body(*args):
        operands = list(args)
        if partition_name is not None:
            operands.append(bass2jax.partition_id_tensor())
        outs = bass2jax._bass_exec_p.bind(
            *operands,
            out_avals=tuple(out_avals),
            in_names=tuple(all_names),
            out_names=tuple(out_names),
            lowering_input_output_aliases=(),
            sim_require_finite=True,
            sim_require_nnan=True,
            nc=nc,
        )
        return tuple(outs)

    devices = jax.devices()[:n_cores]
    mesh = Mesh(np.asarray(devices), ("core",))
    nout = len(out_names)
    donate = tuple(range(n_params, n_params + nout))
    sharded = jax.jit(
        shard_map(
            _body,
            mesh=mesh,
            in_specs=(PartitionSpec("core"),) * (n_params + nout),
            out_specs=(PartitionSpec("core"),) * nout,
            check_rep=False,
        ),
        donate_argnums=donate,
        keep_unused=True,
    )
    sh = NamedSharding(mesh, PartitionSpec("core"))
    dev_in = [
        jax.device_put(
            np.concatenate([np.asarray(m[name]) for m in in_maps], axis=0), sh
        )
        for name in in_names
    ]
    concat_zero_shapes = [
        ((n_cores * z.shape[0], *z.shape[1:]), z.dtype) for z in zero_outs
    ]

    def call():
        zs = [np.zeros(s, d) for s, d in concat_zero_shapes]
        outs = sharded(*dev_in, *zs)
        for o in outs:
            o.block_until_ready()
        return outs

    return call


def bench_loop(hidden_states, Wq, bq, Wk, bk, lengths, reps=(1, 11, 51), iters=6):
    """Estimate device exec time by running the NEFF `n` times inside one
    dispatch for several n and fitting the slope (ns per execution)."""
    import time

    import jax
    from jax.experimental.shard_map import shard_map
    from jax.sharding import Mesh, NamedSharding, PartitionSpec

    import concourse.mybir as mybir_
    from concourse import bass2jax

    nc, _ = _get_program()
    in_maps = _make_in_maps(hidden_states, Wq, bq, Wk, bk, lengths)
    bass2jax.install_neuronx_cc_hook()
    n_cores = len(in_maps)
    partition_name = nc.partition_id_tensor.name if nc.partition_id_tensor else None
    in_names, out_names, out_avals = [], [], []
    for alloc in nc.m.functions[0].allocations:
        if not isinstance(alloc, mybir_.MemoryLocationSet) or not alloc.memorylocations:
            continue
        name = alloc.memorylocations[0].name
        if alloc.kind == "ExternalInput":
            if name != partition_name:
                in_names.append(name)
        elif alloc.kind == "ExternalOutput":
            out_names.append(name)
            out_avals.append(
                jax.core.ShapedArray(tuple(alloc.tensor_shape), mybir_.dt.np(alloc.dtype))
            )
    all_names = in_names + out_names
    if partition_name is not None:
        all_names = all_names + [partition_name]

    devices = jax.devices()[:n_cores]
    mesh = Mesh(np.asarray(devices), ("core",))
    sh = NamedSharding(mesh, PartitionSpec("core"))
    dev_in = [
        jax.device_put(
            np.concatenate([np.asarray(m[name]) for m in in_maps], axis=0), sh
        )
        for name in in_names
    ]
    dev_in += [
        jax.device_put(
            np.zeros((n_cores * a.shape[0], *a.shape[1:]), a.dtype), sh
        )
        for a in out_avals
    ]

    nin = len(in_names)
    nout = len(out_names)

    def make_fn(n):
        def body_n(*args):
            ins, zs = args[:nin], args[nin:]
            outs = None
            for _ in range(n):
                operands = list(ins) + list(zs)
                if partition_name is not None:
                    operands.append(bass2jax.partition_id_tensor())
                outs = bass2jax._bass_exec_p.bind(
                    *operands,
                    out_avals=tuple(out_avals),
                    in_names=tuple(all_names),
                    out_names=tuple(out_names),
                    lowering_input_output_aliases=(),
                    sim_require_finite=True,
                    sim_require_nnan=True,
                    nc=nc,
                )
            return tuple(outs)

        return jax.jit(
            shard_map(
                body_n,
                mesh=mesh,
                in_specs=(PartitionSpec("core"),) * (nin + nout),
                out_specs=(PartitionSpec("core"),) * nout,
                check_rep=False,
            )
        )

    results = {}
    for n in reps:
        fn = make_fn(n)
        outs = fn(*dev_in)
        for o in outs:
            o.block_until_ready()
        ts = []
        for _ in range(iters):
            t0 = time.perf_counter()
            outs = fn(*dev_in)
            for o in outs:
                o.block_until_ready()
            ts.append(time.perf_counter() - t0)
        results[n] = min(ts)
    ns = sorted(results)
    slope = (results[ns[-1]] - results[ns[0]]) / (ns[-1] - ns[0])
    return results, slope


def bench(hidden_states, Wq, bq, Wk, bk, lengths, iters=20):
    """Returns (list of per-iter wall seconds, overhead estimate seconds)."""
    import time

    nc, _ = _get_program()
    in_maps = _make_in_maps(hidden_states, Wq, bq, Wk, bk, lengths)
    call = _make_sharded_callable(nc, in_maps)
    call()  # warm/compile
    times = []
    for _ in range(iters):
        t0 = time.perf_counter()
        call()
        times.append(time.perf_counter() - t0)

    # dispatch-overhead floor: trivial kernel doing one small DMA
    if "nc_trivial" not in _CACHE:
        ncT = bacc.Bacc(None, target_bir_lowering=False, debug=False)
        a = ncT.dram_tensor("a", [BPC, H], F32, kind="ExternalInput").ap()
        yT = ncT.dram_tensor("y", [BPC, H], F32, kind="ExternalOutput").ap()
        with tile.TileContext(ncT) as tcT:
            with tcT.tile_pool(name="p", bufs=1) as pool:
                tt = pool.tile([BPC, H], F32)
                ncT.sync.dma_start(tt, a)
                ncT.sync.dma_start(yT, tt)
        ncT.finalize()
        _CACHE["nc_trivial"] = ncT
    ncT = _CACHE["nc_trivial"]
    triv_maps = [{"a": np.zeros((BPC, H), np.float32)} for _ in range(NCORES)]
    tcall = _make_sharded_callable(ncT, triv_maps)
    tcall()
    otimes = []
    for _ in range(iters):
        t0 = time.perf_counter()
        tcall()
        otimes.append(time.perf_counter() - t0)
    return times, min(otimes)



# revision 3
# speedup vs baseline: 1.1601x; 1.1601x over previous
"""Trainium2 Bass kernel for nn_Attention_82660940579436.

Computation (see reference):
    q     = mean_s(hidden @ Wq.T + bq)            [B, H]
    key   = tanh(hidden @ Wk.T + bk)              [S, B, H]
    score = einsum('bsh,bh->bs', key, q) + mask   [B, S]
    out   = softmax(score) @ key                  [B, H]

Sharding: data-parallel over batch. B=32 over 8 cores -> 4 batches/core.

v3 design (vs the 290us v2 baseline):
  - The hidden slice is pre-transposed and bf16-cast ON HOST into the exact
    SBUF layout the key matmuls need: [chunk, j_local, (tile, cj, tok)],
    tok = g*32 + s_local (g-major).  This removes all PE transposes and the
    per-tile PSUM->SBUF copy (50us of DVE) the old kernel used to
    manufacture hT on-chip.
  - Hidden is DMA'd in 16-tile chunks (8 SWDGE loads instead of 128):
    SWDGE descriptor-gen costs ~1us of Pool-engine time PER DMA, so the old
    per-tile loads burned 133us of Pool; chunking cuts that to ~8us.
  - g-major tok makes the phase-A mean-reduce's innermost axis stride-1, so
    the bf16 reduce can use the DVE 2x/4x perf modes.
  - Bias bk is added by PE (rank-1 matmul) on ~60% of tiles and by a DVE
    in-place PSUM add on the rest, balancing the two phase-A bottlenecks.
  - Phase B scores: prod on DVE (70%) / GPSIMD (30%); row-sum on DVE (70%)
    / ACT accum (30%); exp+mask on ACT; numer/den accumulate on PE.

Device algorithm per core (4 local batches, tok = (g, s_local)):
  Phase A per tile: key_ps[tok,i] = bk + sum_cj hT_cj.T @ Wk_cj (PE, bf16);
    key = tanh(key_ps) -> resident SBUF bf16; red = sum_s hT (DVE, bf16);
    macc += red (fp32).
  q = (macc/S) @ WqT + bq; qrep[p,:] = q[p//32,:] via PE.
  Phase B per tile: sc[p] = sum_i key[p,i]*qrep[p,i] (DVE/Pool mul +
    DVE/ACT reduce); e = exp(sc + mask_bias) (ACT); ei[p,g] = e[p]*(p//32==g);
    numer[4,512] += ei.T @ key ; den[4,1] += ei.T @ ones (PE).
  out = numer / den.

exp() needs no max-subtraction: scores are O(1) by construction, masked
positions get -60 bias -> exp underflows to ~1e-27 (reference's -10000
mask likewise produces exact zeros after its own softmax).

All constants ship in two packed tensors (one fp32, one bf16) loaded by a
single DMA each; two dummy PE ops observe those DMA lanes up front so no
real matmul needs two sync-waits (walrus allows one on a Matmult).
"""

import sys
from contextlib import ExitStack

import numpy as np

if "/opt/trn_rl_repo" not in sys.path:
    sys.path.insert(0, "/opt/trn_rl_repo")

import ml_dtypes  # noqa: E402

import concourse.bacc as bacc  # noqa: E402
import concourse.bass as bass  # noqa: E402
import concourse.mybir as mybir  # noqa: E402
import concourse.tile as tile  # noqa: E402
from concourse.bass_utils import run_bass_kernel_spmd  # noqa: E402

S, B, H = 4096, 32, 512
NCORES = 8
BPC = B // NCORES  # 4 batches per core
NT = 128  # tiles per core
SS = S // NT  # 32 s-positions per tile
TOK = SS * BPC  # 128 tokens per tile; tok = g*SS + s_local (g-major)
HC = H // 128  # 4 chunks of the H (j / i) dims
CHUNK = 16  # tiles per hidden DMA
NCHUNK = NT // CHUNK
MASK_NEG = -60.0
F32 = mybir.dt.float32
BF16 = mybir.dt.bfloat16
AF = mybir.ActivationFunctionType
ALU = mybir.AluOpType
BF16NP = ml_dtypes.bfloat16

# fp32 const pack layout (offsets in fp32 elements, [128, PACKF] tensor)
OFF_MASK = 0  # [128, NT] mask bias (0 / MASK_NEG), col=tile
OFF_BQ = 128  # [4, 512] bq rows
OFF_ZERO = 640  # [128, 1] zeros (tanh bias)
PACKF = 641
# bf16 const pack layout ([128, PACKB]) — matmul operands live here:
# fp32 matmuls run at 1/4 rate on TRN2, bf16 at full rate.
OFFB_WK = 0  # [128, 2048] WkT chunks
OFFB_WQ = 2048  # [128, 2048] WqT chunks
OFFB_BKB = 4096  # [128, 512] bk broadcast to all partitions (DVE bias add)
OFFB_BK = 4608  # [1, 512] bk on partition 0 (PE bias matmul rhs)
OFFB_ONESROW = 5120  # [1, 128] ones on partition 0 (PE bias matmul lhsT)
OFFB_IND4 = 5248  # [128, 4] indicator (p//SS == g)
OFFB_IND4T = 5252  # [4, 128] indicator transposed
OFFB_ONES = 5380  # [128, 1] ones
PACKB = 5382

# tuning knobs (read at build time)
KNOBS = {
    "h_bufs": 2,
    "keyps_bufs": 5,
    "small_bufs": 4,
    # phase A: bias via PE rank-1 matmul when True else DVE in-place PSUM add
    "bias_pe": lambda t: t % 5 < 3,
    # phase B: score product on DVE vs GPSIMD; row-sum on DVE vs ACT accum
    "mul_dve": lambda t: t % 10 < 7,
    "red_dve": lambda t: t % 10 < 7,
}


def _build_kernel_body(tc, aps):
    nc = tc.nc
    x, packf, packb, y = aps["x"], aps["packf"], aps["packb"], aps["y"]

    with ExitStack() as ctx:
        consts = ctx.enter_context(tc.tile_pool(name="consts", bufs=1))
        ph = ctx.enter_context(tc.tile_pool(name="h", bufs=KNOBS["h_bufs"]))
        pkeys = ctx.enter_context(tc.tile_pool(name="keys", bufs=NT))
        psmall = ctx.enter_context(tc.tile_pool(name="small", bufs=KNOBS["small_bufs"]))
        pacc = ctx.enter_context(tc.tile_pool(name="acc", bufs=1))
        pps_key = ctx.enter_context(
            tc.tile_pool(name="ps_key", bufs=KNOBS["keyps_bufs"], space="PSUM")
        )
        pps_acc = ctx.enter_context(tc.tile_pool(name="ps_acc", bufs=1, space="PSUM"))
        pps_sm = ctx.enter_context(tc.tile_pool(name="ps_sm", bufs=1, space="PSUM"))

        # ---- constants: one DMA per pack ----
        cf = consts.tile([128, PACKF], F32)
        nc.sync.dma_start(cf, packf)
        cb = consts.tile([128, PACKB], BF16)
        nc.sync.dma_start(cb, packb)

        def wk_sb(c):
            return cb[:, OFFB_WK + c * 512 : OFFB_WK + (c + 1) * 512]

        def wq_sb(c):
            return cb[:, OFFB_WQ + c * 512 : OFFB_WQ + (c + 1) * 512]

        maskb_sb = cf[:, OFF_MASK : OFF_MASK + NT]
        bq_sb = cf[0:BPC, OFF_BQ : OFF_BQ + H]
        zero_sb = cf[:, OFF_ZERO : OFF_ZERO + 1]
        bkb_sb = cb[:, OFFB_BKB : OFFB_BKB + H]
        bk_sb = cb[0:1, OFFB_BK : OFFB_BK + H]
        ones_row_sb = cb[0:1, OFFB_ONESROW : OFFB_ONESROW + 128]
        ind4_sb = cb[:, OFFB_IND4 : OFFB_IND4 + BPC]
        ind4T_sb = cb[0:BPC, OFFB_IND4T : OFFB_IND4T + 128]
        ones1_sb = cb[:, OFFB_ONES : OFFB_ONES + 1]

        # Dummy PE ops: observe each const-pack DMA lane once, so no real
        # matmul ever needs two sync-waits (walrus S3_LW limit is one).
        scr = pps_sm.tile([128, H], F32, tag="sm")
        nc.tensor.matmul(scr[0:BPC, 0:BPC], ind4_sb, ind4_sb, start=True, stop=True)
        nc.tensor.matmul(scr[0:1, 0:1], zero_sb, zero_sb, start=True, stop=True)

        macc = pacc.tile([128, HC * BPC], F32)  # sum_s h, laid out (j_local, (c, g))
        nc.vector.memset(macc, 0.0)

        # ---- Phase A ----
        keys = []
        for ch in range(NCHUNK):
            h_t = ph.tile([128, CHUNK * H], BF16, tag="h")
            nc.gpsimd.dma_start(h_t, x[ch])
            for tl in range(CHUNK):
                t = ch * CHUNK + tl
                hview = h_t[:, tl * H : (tl + 1) * H]
                key_ps = pps_key.tile([TOK, H], F32, tag="key")
                bias_pe = KNOBS["bias_pe"](t)
                if bias_pe:
                    nc.tensor.matmul(key_ps, ones_row_sb, bk_sb, start=True, stop=False)
                for c in range(HC):
                    nc.tensor.matmul(
                        key_ps,
                        hview[:, c * 128 : (c + 1) * 128],
                        wk_sb(c),
                        start=(c == 0 and not bias_pe),
                        stop=(c == HC - 1),
                    )
                if not bias_pe:
                    nc.vector.tensor_add(key_ps, key_ps, bkb_sb)
                key_t = pkeys.tile([TOK, H], BF16, tag="key")
                nc.scalar.activation(key_t, key_ps, AF.Tanh, bias=zero_sb)
                keys.append(key_t)

                red = psmall.tile([128, HC * BPC], BF16, tag="red")
                # bf16 out keeps the DVE 2x/4x perf mode; 32-term tile sums
                # round at ~1e-2 and the fp32 macc accumulation averages the
                # error down across 128 tiles (q tolerance is loose).
                with nc.allow_low_precision(reason="tile-local 32-term sum"):
                    nc.vector.tensor_reduce(
                        red,
                        hview.rearrange("p (c g s) -> p c g s", c=HC, g=BPC, s=SS),
                        axis=mybir.AxisListType.X,
                        op=ALU.add,
                    )
                nc.vector.tensor_add(macc, macc, red)

        # ---- q = (sum_s h / S) @ WqT + bq ; qrep[p] = q[p//SS] ----
        maccb = pacc.tile([128, HC * BPC], BF16)
        nc.vector.tensor_copy(maccb, macc)
        q_ps = pps_sm.tile([128, H], F32, tag="sm")
        for c in range(HC):
            nc.tensor.matmul(
                q_ps[0:BPC, :],
                maccb[:, c * BPC : (c + 1) * BPC],
                wq_sb(c),
                start=(c == 0),
                stop=(c == HC - 1),
            )
        q_sb = pacc.tile([BPC, H], F32)
        nc.scalar.mul(q_sb, q_ps[0:BPC, :], 1.0 / S)
        q_b = pacc.tile([BPC, H], BF16)
        nc.vector.tensor_add(q_b, q_sb, bq_sb)
        qrep_ps = pps_sm.tile([128, H], F32, tag="sm")
        nc.tensor.matmul(qrep_ps, ind4T_sb, q_b, start=True, stop=True)
        qrep_sb = pacc.tile([128, H], BF16)
        nc.vector.tensor_copy(qrep_sb, qrep_ps)

        # ---- Phase B ----
        numer_ps = pps_acc.tile([BPC, H], F32, tag="numer")
        den_ps = pps_acc.tile([BPC, 1], F32, tag="den")
        for t in range(NT):
            if KNOBS["mul_dve"](t):
                prod = psmall.tile([TOK, H], BF16, tag="prod")
                nc.vector.tensor_mul(prod, keys[t], qrep_sb)
            else:
                prod = psmall.tile([TOK, H], BF16, tag="prodg")
                nc.gpsimd.tensor_mul(prod, keys[t], qrep_sb)
            sc_t = psmall.tile([TOK, 1], F32, tag="sc")
            if KNOBS["red_dve"](t):
                nc.vector.tensor_reduce(
                    sc_t, prod, axis=mybir.AxisListType.X, op=ALU.add
                )
            else:
                pc = psmall.tile([TOK, H], BF16, tag="pc")
                nc.scalar.activation(pc, prod, AF.Copy, accum_out=sc_t)
            e_t = psmall.tile([TOK, 1], F32, tag="e")
            nc.scalar.activation(e_t, sc_t, AF.Exp, bias=maskb_sb[:, t : t + 1])
            ei_t = psmall.tile([TOK, BPC], BF16, tag="ei")
            nc.vector.tensor_scalar_mul(ei_t, ind4_sb, e_t)
            nc.tensor.matmul(
                numer_ps, ei_t, keys[t], start=(t == 0), stop=(t == NT - 1)
            )
            nc.tensor.matmul(
                den_ps, ei_t, ones1_sb, start=(t == 0), stop=(t == NT - 1)
            )

        # ---- out = numer / den ----
        rcp = pacc.tile([BPC, 1], F32)
        nc.vector.reciprocal(rcp, den_ps)
        out_sb = pacc.tile([BPC, H], F32)
        nc.vector.tensor_scalar_mul(out_sb, numer_ps, rcp)
        nc.sync.dma_start(y, out_sb)


_CACHE = {}


def _get_program():
    if "nc" in _CACHE:
        return _CACHE["nc"], _CACHE["aps"]
    nc = bacc.Bacc(None, target_bir_lowering=False, debug=False)
    aps = {
        "x": nc.dram_tensor(
            "x", [NCHUNK, 128, CHUNK * H], BF16, kind="ExternalInput"
        ).ap(),
        "packf": nc.dram_tensor("packf", [128, PACKF], F32, kind="ExternalInput").ap(),
        "packb": nc.dram_tensor("packb", [128, PACKB], BF16, kind="ExternalInput").ap(),
        "y": nc.dram_tensor("y", [BPC, H], F32, kind="ExternalOutput").ap(),
    }
    with tile.TileContext(nc) as tc:
        _build_kernel_body(tc, aps)
    nc.finalize()  # Bacc.compile: wait legalization (EVSEM splits), LDW moves
    _CACHE["nc"] = nc
    _CACHE["aps"] = aps
    return nc, aps


def _make_in_maps(hidden_states, Wq, bq, Wk, bk, lengths):
    hidden = np.asarray(hidden_states, dtype=np.float32)
    Wq = np.asarray(Wq, dtype=np.float32)
    Wk = np.asarray(Wk, dtype=np.float32)
    bqv = np.asarray(bq, dtype=np.float32)
    bkv = np.asarray(bk, dtype=np.float32)
    lens = np.asarray(lengths).astype(np.int64)

    p = np.arange(128)
    packb = np.zeros((128, PACKB), dtype=BF16NP)
    packb[:, OFFB_WK : OFFB_WK + 2048] = (
        np.ascontiguousarray(Wk.T)
        .reshape(HC, 128, H)
        .transpose(1, 0, 2)
        .reshape(128, 2048)
        .astype(BF16NP)
    )
    packb[:, OFFB_WQ : OFFB_WQ + 2048] = (
        np.ascontiguousarray(Wq.T)
        .reshape(HC, 128, H)
        .transpose(1, 0, 2)
        .reshape(128, 2048)
        .astype(BF16NP)
    )
    packb[:, OFFB_BKB : OFFB_BKB + H] = bkv[None, :].astype(BF16NP)
    packb[0, OFFB_BK : OFFB_BK + H] = bkv.astype(BF16NP)
    packb[0, OFFB_ONESROW : OFFB_ONESROW + 128] = BF16NP(1.0)
    packb[:, OFFB_IND4 : OFFB_IND4 + BPC] = (
        p[:, None] // SS == np.arange(BPC)[None, :]
    ).astype(BF16NP)
    packb[0:BPC, OFFB_IND4T : OFFB_IND4T + 128] = (
        p[None, :] // SS == np.arange(BPC)[:, None]
    ).astype(BF16NP)
    packb[:, OFFB_ONES] = BF16NP(1.0)

    base_packf = np.zeros((128, PACKF), dtype=np.float32)
    base_packf[0:BPC, OFF_BQ : OFF_BQ + H] = bqv[None, :]

    in_maps = []
    s_of_p = p % SS
    t_idx = np.arange(NT)
    for c in range(NCORES):
        hc = hidden[:, c * BPC : (c + 1) * BPC, :]  # [S, 4, 512]
        # -> [chunk, j_local, (tile, cj, g, s_local)] bf16
        xc = np.ascontiguousarray(
            hc.reshape(NCHUNK, CHUNK, SS, BPC, HC, 128).transpose(0, 5, 1, 4, 3, 2)
        ).reshape(NCHUNK, 128, CHUNK * H).astype(BF16NP)
        packf = base_packf.copy()
        b_of_p = c * BPC + (p // SS)
        s_full = SS * t_idx[None, :] + s_of_p[:, None]  # [128, NT]
        valid = s_full < lens[b_of_p][:, None]
        packf[:, OFF_MASK : OFF_MASK + NT] = np.where(valid, 0.0, MASK_NEG)
        in_maps.append({"x": xc, "packf": packf, "packb": packb})
    return in_maps


def run(hidden_states, Wq, bq, Wk, bk, lengths, trace=False):
    """Run on 8 cores; returns (output [B, H] fp32, BassKernelResults)."""
    nc, _ = _get_program()
    in_maps = _make_in_maps(hidden_states, Wq, bq, Wk, bk, lengths)
    res = run_bass_kernel_spmd(
        nc, in_maps, core_ids=list(range(NCORES)), trace=trace
    )
    out = np.concatenate([np.asarray(r["y"]) for r in res.results], axis=0)
    return out.astype(np.float32), res


def kernel(hidden_states, Wq, bq, Wk, bk, lengths):
    out, _ = run(hidden_states, Wq, bq, Wk, bk, lengths)
    return out


# revision 16
# speedup vs baseline: 1.2943x; 1.1157x over previous
"""Trainium2 Bass kernel for nn_Attention_82660940579436.

Computation (see reference):
    q     = mean_s(hidden @ Wq.T + bq)            [B, H]
    key   = tanh(hidden @ Wk.T + bk)              [S, B, H]
    score = einsum('bsh,bh->bs', key, q) + mask   [B, S]
    out   = softmax(score) @ key                  [B, H]

Sharding: data-parallel over batch. B=32 over 8 cores -> 4 batches/core.

v4 design (290us baseline -> 250us v3 -> this):
  - Hidden is pre-transposed and bf16-cast ON HOST into the exact SBUF
    layout the key matmuls need ([chunk, j_local, (tile, cj, s)]), so no
    PE transposes / PSUM->SBUF copies are needed on-chip, and is DMA'd in
    16-tile chunks (8 SWDGE loads instead of 128; descriptor generation
    costs ~1us of Pool time per load).
  - One batch per 128-token tile (4 groups of 32 tiles, tok = s_local).
    Phase A of group g (PE-bound: key projection) is interleaved with
    phase B of group g-1 (DVE/ACT/Pool-bound: softmax+weighted sum), so
    the per-phase bottleneck engines overlap.  With all 4 batches
    interleaved in a tile (v3) the phases were strictly sequential
    because scores need q = mean over the FULL sequence.
  - One batch per tile also kills the per-tile indicator multiply: the
    exp() output itself (bf16) is the lhsT of the numer/den matmuls.
  - Work is split across engines by knob: bias bk via PE rank-1 matmul /
    DVE in-place PSUM add; score product on DVE / GPSIMD; score row-sum
    on DVE / ACT accumulate.  TensorReduce gets no DVE 2x perf mode, so
    reductions stay fp32 (same cost, better accuracy).

Device algorithm per core:
  Phase A per tile (128 s-positions of one batch g):
    key_ps[s,i] = bk + sum_cj hT_cj.T @ Wk_cj  (PE, bf16 -> PSUM fp32)
    key = tanh(key_ps) -> resident SBUF bf16
    red[j, c] = sum_s hT  (DVE fp32);  macc[:, g] += red
  q_g = (macc_g/S) @ WqT + bq;  qrep_g[p,:] = q_g (PE broadcast matmul)
  Phase B per tile: sc[s] = sum_i key[s,i]*qrep_g[s,i] (DVE/Pool mul +
    DVE/ACT reduce); e = exp(sc + mask_bias) bf16 (ACT);
    numer[g,:] += e.T @ key ; den[g] += e.T @ ones  (PE).
  out = numer / den.

exp() needs no max-subtraction: scores are O(1) by construction, masked
positions get -60 bias -> exp underflows to ~1e-27 (reference's -10000
mask likewise produces exact zeros after its own softmax).

All constants ship in two packed tensors (one fp32, one bf16) loaded by a
single DMA each; two dummy PE ops observe those DMA lanes up front so no
real matmul needs two sync-waits (walrus allows one on a Matmult).
"""

import sys
from contextlib import ExitStack

import numpy as np

if "/opt/trn_rl_repo" not in sys.path:
    sys.path.insert(0, "/opt/trn_rl_repo")

import ml_dtypes  # noqa: E402

import concourse.bacc as bacc  # noqa: E402
import concourse.bass as bass  # noqa: E402
import concourse.mybir as mybir  # noqa: E402
import concourse.tile as tile  # noqa: E402
from concourse.bass_utils import run_bass_kernel_spmd  # noqa: E402

S, B, H = 4096, 32, 512
NCORES = 8
BPC = B // NCORES  # 4 batches per core = 4 groups
NT = 128  # tiles per core
TPG = NT // BPC  # 32 tiles per group
TOK = S // TPG  # 128 tokens (s-positions) per tile
HC = H // 128  # 4 chunks of the H (j / i) dims
CHUNK = 16  # tiles per hidden DMA
NCHUNK = NT // CHUNK
MASK_NEG = -60.0
F32 = mybir.dt.float32
BF16 = mybir.dt.bfloat16
AF = mybir.ActivationFunctionType
ALU = mybir.AluOpType
BF16NP = ml_dtypes.bfloat16

# fp32 const pack layout (offsets in fp32 elements, [128, PACKF] tensor)
OFF_MASK = 0  # [128, NT] mask bias (0 / MASK_NEG), col=global tile
OFF_BQ = 128  # [1, 512] bq row
OFF_ZERO = 640  # [128, 1] zeros (tanh bias)
PACKF = 641
# bf16 const pack layout ([128, PACKB]) — matmul operands live here:
# fp32 matmuls run at 1/4 rate on TRN2, bf16 at full rate.
OFFB_WK = 0  # [128, 2048] WkT chunks
OFFB_WQ = 2048  # [128, 2048] WqT chunks
OFFB_BKB = 4096  # [128, 512] bk broadcast to all partitions (DVE bias add)
OFFB_BK = 4608  # [1, 512] bk on partition 0 (PE bias matmul rhs)
OFFB_ONESROW = 5120  # [1, 128] ones on partition 0 (PE bias/bcast lhsT)
OFFB_ONES = 5248  # [128, 1] ones
OFFB_OH = 5250  # [128, 4*4] one-hot rows: cols g*4+g' = (g == g')
PACKB = 5266

# tuning knobs (read at build time)
KNOBS = {
    "h_bufs": 2,
    "keyps_bufs": 5,
    "small_bufs": 4,
    "stagger": 4,  # tiles of A(g) emitted before B(g-1) starts
    # phase A: bias via PE rank-1 matmul when True else DVE in-place PSUM add.
    # During the first group (T<32) no phase-B work overlaps, so PE is the
    # only bottleneck and more bias goes to DVE.
    "bias_pe": lambda t: (t % 2 == 0) if t < 32 else (t % 3 == 0),
    # phase B: fused score mul+rowsum (scalar_tensor_tensor) on DVE when True,
    # else plain mul on GPSIMD + rowsum on ACT accumulate (TSP-with-accum is
    # not a valid Pool opcode)
    "stt_dve": lambda t: True,
}


def _build_kernel_body(tc, aps):
    nc = tc.nc
    x, packf, packb, y = aps["x"], aps["packf"], aps["packb"], aps["y"]

    with ExitStack() as ctx:
        consts = ctx.enter_context(tc.tile_pool(name="consts", bufs=1))
        ph = ctx.enter_context(tc.tile_pool(name="h", bufs=KNOBS["h_bufs"]))
        pkeys = ctx.enter_context(tc.tile_pool(name="keys", bufs=NT))
        psmall = ctx.enter_context(tc.tile_pool(name="small", bufs=KNOBS["small_bufs"]))
        pacc = ctx.enter_context(tc.tile_pool(name="acc", bufs=1))
        pps_key = ctx.enter_context(
            tc.tile_pool(name="ps_key", bufs=KNOBS["keyps_bufs"], space="PSUM")
        )
        pps_acc = ctx.enter_context(tc.tile_pool(name="ps_acc", bufs=1, space="PSUM"))
        pps_sm = ctx.enter_context(tc.tile_pool(name="ps_sm", bufs=1, space="PSUM"))

        # ---- constants: one DMA per pack ----
        cf = consts.tile([128, PACKF], F32)
        nc.sync.dma_start(cf, packf)
        cb = consts.tile([128, PACKB], BF16)
        nc.sync.dma_start(cb, packb)

        def wk_sb(c):
            return cb[:, OFFB_WK + c * 512 : OFFB_WK + (c + 1) * 512]

        def wq_sb(c):
            return cb[:, OFFB_WQ + c * 512 : OFFB_WQ + (c + 1) * 512]

        maskb_sb = cf[:, OFF_MASK : OFF_MASK + NT]
        bq_sb = cf[0:1, OFF_BQ : OFF_BQ + H]
        zero_sb = cf[:, OFF_ZERO : OFF_ZERO + 1]
        bkb_sb = cb[:, OFFB_BKB : OFFB_BKB + H]
        bk_sb = cb[0:1, OFFB_BK : OFFB_BK + H]
        ones_row_sb = cb[0:1, OFFB_ONESROW : OFFB_ONESROW + 128]
        ones1_sb = cb[:, OFFB_ONES : OFFB_ONES + 1]

        def oh_sb(g):
            return cb[:, OFFB_OH + g * BPC : OFFB_OH + (g + 1) * BPC]

        # Dummy PE ops: observe each const-pack DMA lane once, so no real
        # matmul ever needs two sync-waits (walrus S3_LW limit is one).
        scr = pps_sm.tile([128, H], F32, tag="sm")
        nc.tensor.matmul(scr[0:1, 0:1], ones1_sb, ones1_sb, start=True, stop=True)
        nc.tensor.matmul(scr[0:1, 0:1], zero_sb, zero_sb, start=True, stop=True)

        # per-tile partial sums of h: red_all[j, T*HC + c] = sum_s hT (fp32,
        # written by the TSP accumulators; reduced per group in emit_q)
        red_all = pacc.tile([128, NT * HC], F32)

        keys = [None] * NT
        chunk_tiles = [None] * NCHUNK
        qrep = [None] * BPC
        numer_ps = pps_acc.tile([BPC, H], F32, tag="numer")
        den_ps = pps_acc.tile([BPC, 1], F32, tag="den")

        def emit_a(g, t):
            T = g * TPG + t
            ch, tl = divmod(T, CHUNK)
            if tl == 0:
                h_t = ph.tile([128, CHUNK * H], BF16, tag="h")
                nc.gpsimd.dma_start(h_t, x[ch])
                chunk_tiles[ch] = h_t
            hview = chunk_tiles[ch][:, tl * H : (tl + 1) * H]
            key_ps = pps_key.tile([TOK, H], F32, tag="key")
            bias_pe = KNOBS["bias_pe"](T)
            if bias_pe:
                nc.tensor.matmul(key_ps, ones_row_sb, bk_sb, start=True, stop=False)
            for c in range(HC):
                nc.tensor.matmul(
                    key_ps,
                    hview[:, c * 128 : (c + 1) * 128],
                    wk_sb(c),
                    start=(c == 0 and not bias_pe),
                    stop=(c == HC - 1),
                )
            if not bias_pe:
                nc.vector.tensor_add(key_ps, key_ps, bkb_sb)
            key_t = pkeys.tile([TOK, H], BF16, tag="key")
            nc.scalar.activation(key_t, key_ps, AF.Tanh, bias=zero_sb)
            keys[T] = key_t

            # per-(j, c) partial sums over s via TSP accumulate (out is a
            # throwaway copy; the fp32 accum_out is the real product)
            for c in range(HC):
                pd = psmall.tile([128, 128], BF16, tag="pd")
                nc.vector.tensor_scalar(
                    pd,
                    hview[:, c * 128 : (c + 1) * 128],
                    1.0,
                    0.0,
                    op0=ALU.mult,
                    op1=ALU.add,
                    accum_out=red_all[:, T * HC + c : T * HC + c + 1],
                )

        def emit_q(g):
            redg = pacc.tile([128, HC], F32, tag=f"rg{g}")
            nc.vector.tensor_reduce(
                redg,
                red_all[
                    :, g * TPG * HC : (g + 1) * TPG * HC
                ].rearrange("p (t c) -> p c t", t=TPG, c=HC),
                axis=mybir.AxisListType.X,
                op=ALU.add,
            )
            maccb = pacc.tile([128, HC], BF16, tag=f"mb{g}")
            nc.vector.tensor_copy(maccb, redg)
            q_ps = pps_sm.tile([128, H], F32, tag="sm")
            for c in range(HC):
                nc.tensor.matmul(
                    q_ps[0:1, :],
                    maccb[:, c : c + 1],
                    wq_sb(c),
                    start=(c == 0),
                    stop=(c == HC - 1),
                )
            q_sb = pacc.tile([1, H], F32, tag=f"q{g}")
            nc.scalar.mul(q_sb, q_ps[0:1, :], 1.0 / S)
            q_b = pacc.tile([1, H], BF16, tag=f"qb{g}")
            nc.vector.tensor_add(q_b, q_sb, bq_sb)
            qrep_ps = pps_sm.tile([128, H], F32, tag="sm")
            nc.tensor.matmul(qrep_ps, ones_row_sb, q_b, start=True, stop=True)
            qrep_g = pacc.tile([128, H], BF16, tag=f"qr{g}")
            nc.vector.tensor_copy(qrep_g, qrep_ps)
            qrep[g] = qrep_g

        def emit_b(g, t):
            T = g * TPG + t
            key_t = keys[T]
            sc_t = psmall.tile([TOK, 1], F32, tag="sc")
            if KNOBS["stt_dve"](T):
                prod = psmall.tile([TOK, H], BF16, tag="prod")
                nc.vector.scalar_tensor_tensor(
                    prod, key_t, 1.0, qrep[g], ALU.mult, ALU.mult, accum_out=sc_t
                )
            else:
                prod = psmall.tile([TOK, H], BF16, tag="prodg")
                nc.gpsimd.tensor_mul(prod, key_t, qrep[g])
                pc = psmall.tile([TOK, H], BF16, tag="pc")
                nc.scalar.activation(pc, prod, AF.Copy, accum_out=sc_t)
            e_b = psmall.tile([TOK, 1], F32, tag="e")
            nc.scalar.activation(e_b, sc_t, AF.Exp, bias=maskb_sb[:, T : T + 1])
            # ei[s, g'] = e[s] * (g' == g): lets one [4, H] PSUM accumulator
            # collect all four groups (matmul outs must start at partition 0)
            ei_t = psmall.tile([TOK, BPC], BF16, tag="ei")
            nc.vector.tensor_scalar_mul(ei_t, oh_sb(g), e_b)
            first = g == 0 and t == 0
            last = g == BPC - 1 and t == TPG - 1
            nc.tensor.matmul(numer_ps, ei_t, key_t, start=first, stop=last)
            nc.tensor.matmul(den_ps, ei_t, ones1_sb, start=first, stop=last)

        # ---- interleaved schedule: A(g) || B(g-1) ----
        stg = KNOBS["stagger"]
        for g in range(BPC):
            for t in range(TPG):
                emit_a(g, t)
                if g >= 1 and t >= stg:
                    emit_b(g - 1, t - stg)
            emit_q(g)
            if g >= 1:
                for bt in range(TPG - stg, TPG):
                    emit_b(g - 1, bt)
        for bt in range(TPG):
            emit_b(BPC - 1, bt)

        # ---- out = numer / den ----
        rcp = pacc.tile([BPC, 1], F32)
        nc.vector.reciprocal(rcp, den_ps)
        out_sb = pacc.tile([BPC, H], F32)
        nc.vector.tensor_scalar_mul(out_sb, numer_ps, rcp)
        nc.sync.dma_start(y, out_sb)


_CACHE = {}


def _get_program():
    if "nc" in _CACHE:
        return _CACHE["nc"], _CACHE["aps"]
    nc = bacc.Bacc(None, target_bir_lowering=False, debug=False)
    aps = {
        "x": nc.dram_tensor(
            "x", [NCHUNK, 128, CHUNK * H], BF16, kind="ExternalInput"
        ).ap(),
        "packf": nc.dram_tensor("packf", [128, PACKF], F32, kind="ExternalInput").ap(),
        "packb": nc.dram_tensor("packb", [128, PACKB], BF16, kind="ExternalInput").ap(),
        "y": nc.dram_tensor("y", [BPC, H], F32, kind="ExternalOutput").ap(),
    }
    with tile.TileContext(nc) as tc:
        _build_kernel_body(tc, aps)
    nc.finalize()  # Bacc.compile: wait legalization (EVSEM splits), LDW moves
    _CACHE["nc"] = nc
    _CACHE["aps"] = aps
    return nc, aps


def _make_in_maps(hidden_states, Wq, bq, Wk, bk, lengths):
    hidden = np.asarray(hidden_states, dtype=np.float32)
    Wq = np.asarray(Wq, dtype=np.float32)
    Wk = np.asarray(Wk, dtype=np.float32)
    bqv = np.asarray(bq, dtype=np.float32)
    bkv = np.asarray(bk, dtype=np.float32)
    lens = np.asarray(lengths).astype(np.int64)

    p = np.arange(128)
    packb = np.zeros((128, PACKB), dtype=BF16NP)
    packb[:, OFFB_WK : OFFB_WK + 2048] = (
        np.ascontiguousarray(Wk.T)
        .reshape(HC, 128, H)
        .transpose(1, 0, 2)
        .reshape(128, 2048)
        .astype(BF16NP)
    )
    packb[:, OFFB_WQ : OFFB_WQ + 2048] = (
        np.ascontiguousarray(Wq.T)
        .reshape(HC, 128, H)
        .transpose(1, 0, 2)
        .reshape(128, 2048)
        .astype(BF16NP)
    )
    packb[:, OFFB_BKB : OFFB_BKB + H] = bkv[None, :].astype(BF16NP)
    packb[0, OFFB_BK : OFFB_BK + H] = bkv.astype(BF16NP)
    packb[0, OFFB_ONESROW : OFFB_ONESROW + 128] = BF16NP(1.0)
    packb[:, OFFB_ONES] = BF16NP(1.0)
    for g in range(BPC):
        packb[:, OFFB_OH + g * BPC + g] = BF16NP(1.0)

    base_packf = np.zeros((128, PACKF), dtype=np.float32)
    base_packf[0, OFF_BQ : OFF_BQ + H] = bqv

    chpg = TPG // CHUNK  # chunks per group
    in_maps = []
    t_idx = np.arange(NT)
    for c in range(NCORES):
        hc = hidden[:, c * BPC : (c + 1) * BPC, :]  # [S, 4, 512]
        # -> [chunk, j_local, (tile_in_chunk, cj, s_local)] bf16
        xc = np.ascontiguousarray(
            hc.transpose(1, 0, 2)  # [g, S, H]
            .reshape(BPC, chpg, CHUNK, TOK, HC, 128)  # g, ch2, tl, p, cj, j
            .transpose(0, 1, 5, 2, 4, 3)  # g, ch2, j, tl, cj, p
        ).reshape(NCHUNK, 128, CHUNK * H).astype(BF16NP)
        packf = base_packf.copy()
        b_of_t = c * BPC + t_idx // TPG  # [NT]
        s_full = (t_idx % TPG)[None, :] * TOK + p[:, None]  # [128, NT]
        valid = s_full < lens[b_of_t][None, :]
        packf[:, OFF_MASK : OFF_MASK + NT] = np.where(valid, 0.0, MASK_NEG)
        in_maps.append({"x": xc, "packf": packf, "packb": packb})
    return in_maps


def run(hidden_states, Wq, bq, Wk, bk, lengths, trace=False):
    """Run on 8 cores; returns (output [B, H] fp32, BassKernelResults)."""
    nc, _ = _get_program()
    in_maps = _make_in_maps(hidden_states, Wq, bq, Wk, bk, lengths)
    res = run_bass_kernel_spmd(
        nc, in_maps, core_ids=list(range(NCORES)), trace=trace
    )
    out = np.concatenate([np.asarray(r["y"]) for r in res.results], axis=0)
    return out.astype(np.float32), res


def kernel(hidden_states, Wq, bq, Wk, bk, lengths):
    out, _ = run(hidden_states, Wq, bq, Wk, bk, lengths)
    return out


# revision 17
# speedup vs baseline: 2.3568x; 1.8209x over previous
"""Trainium2 Bass kernel for nn_Attention_82660940579436.

Computation (see reference):
    q     = mean_s(hidden @ Wq.T + bq)            [B, H]
    key   = tanh(hidden @ Wk.T + bk)              [S, B, H]
    score = einsum('bsh,bh->bs', key, q) + mask   [B, S]
    out   = softmax(score) @ key                  [B, H]

Sharding: data-parallel over batch. B=32 over 8 cores -> 4 batches/core.

v6 design (290us baseline -> 250 -> 224 -> this):
  - Hidden is pre-transposed and bf16-cast ON HOST into the exact SBUF
    layout the key matmuls need ([chunk, j_local, (tile, cj, s)]), DMA'd
    in 8-tile chunks (SWDGE descriptor-gen costs ~1us of Pool time per
    load, so per-tile loads are out).
  - One batch per 128-token tile: 4 groups of 32 tiles, tok = s_local.
  - MASKED-TILE SKIPPING: positions s >= length only reach the output
    through exp(-60) ~ 1e-26, so tiles entirely past a batch's length
    need no key projection / softmax work.  Only q = mean_s(...) needs
    the full sequence.  Batches are SORTED by length on the host and
    assigned to (core, group) slots so each group's max length across
    the 8 cores (the SPMD program is shared) is minimal; the program is
    compiled per set of group tile-counts (cached).  For uniform random
    lengths this skips ~40% of the expensive work.
  - PREPASS/KEYPASS split: a cheap prepass (chunk DMA + per-chunk sums
    via DVE TSP-accumulate + the tiny q chain) runs one group ahead,
    interleaved into the previous group's keypass, so q(g) is ready
    when keypass(g) starts and phase B (softmax) of each tile follows
    its key projection immediately.  Phase A (PE-bound) and phase B
    (DVE/ACT/Pool-bound) overlap everywhere except a short head.
    Chunks are re-loaded for the keypass (DMA device has headroom;
    SBUF does not - keys alone take up to 128KiB/partition).
  - Work splits: bias bk via PE rank-1 matmul; score mul+rowsum fused
    in one DVE scalar_tensor_tensor (TSP-reduce is DVE-only) with a
    per-window fraction diverted to GPSIMD-mul + ACT-accumulate where
    the prepass loads the DVE.

exp() needs no max-subtraction: scores are O(1) by construction, masked
positions get -60 bias -> exp underflows to ~1e-27 (reference's -10000
mask likewise produces exact zeros after its own softmax).

All constants ship in two packed tensors (one fp32, one bf16) loaded by a
single DMA each; two dummy PE ops observe those DMA lanes up front so no
real matmul needs two sync-waits (walrus allows one on a Matmult).
"""

import sys
from contextlib import ExitStack

import numpy as np

if "/opt/trn_rl_repo" not in sys.path:
    sys.path.insert(0, "/opt/trn_rl_repo")

import ml_dtypes  # noqa: E402

import concourse.bacc as bacc  # noqa: E402
import concourse.bass as bass  # noqa: E402
import concourse.mybir as mybir  # noqa: E402
import concourse.tile as tile  # noqa: E402
from concourse.bass_utils import run_bass_kernel_spmd  # noqa: E402

S, B, H = 4096, 32, 512
NCORES = 8
BPC = B // NCORES  # 4 batches per core = 4 groups
NT = 128  # tiles per core
TPG = NT // BPC  # 32 tiles per group
TOK = S // TPG  # 128 tokens (s-positions) per tile
HC = H // 128  # 4 chunks of the H (j / i) dims
CHUNK = 8  # tiles per hidden DMA
NCHUNK = NT // CHUNK
CPG = TPG // CHUNK  # chunks per group
MASK_NEG = -60.0
F32 = mybir.dt.float32
BF16 = mybir.dt.bfloat16
AF = mybir.ActivationFunctionType
ALU = mybir.AluOpType
BF16NP = ml_dtypes.bfloat16

# fp32 const pack layout (offsets in fp32 elements, [128, PACKF] tensor)
OFF_MASK = 0  # [128, NT] mask bias (0 / MASK_NEG), col=global tile
OFF_BQ = 128  # [1, 512] bq row
OFF_ZERO = 640  # [128, 1] zeros (tanh bias)
PACKF = 641
# bf16 const pack layout ([128, PACKB]) — matmul operands live here:
# fp32 matmuls run at 1/4 rate on TRN2, bf16 at full rate.
OFFB_WK = 0  # [128, 2048] WkT chunks
OFFB_WQ = 2048  # [128, 2048] WqT chunks
OFFB_BK = 4096  # [1, 512] bk on partition 0 (PE bias matmul rhs)
OFFB_ONESROW = 4608  # [1, 128] ones on partition 0 (PE bias/bcast lhsT)
OFFB_ONES = 4736  # [128, 1] ones
OFFB_OH = 4738  # [128, 4*4] one-hot rows: cols g*4+g' = (g == g')
PACKB = 4754

# tuning knobs (read at build time)
KNOBS = {
    "hp_bufs": 2,  # prepass chunk buffers
    "hk_bufs": 2,  # keypass chunk buffers
    "keyps_bufs": 5,
    "small_bufs": 4,
    "stagger": 5,  # tiles of keypass(g) emitted before B(g) starts
    # phase B: fraction (in tenths) of tiles whose fused score mul+rowsum
    # runs on DVE (scalar_tensor_tensor), per window; the rest go to
    # GPSIMD-mul + ACT-accumulate.  Window 2 carries the heaviest prepass
    # red load on DVE; window 3 has no prepass.
    "stt_dve10": [8, 6, 4, 9],
}


def _build_kernel_body(tc, aps, t_cnts):
    nc = tc.nc
    x, packf, packb, y = aps["x"], aps["packf"], aps["packb"], aps["y"]

    with ExitStack() as ctx:
        consts = ctx.enter_context(tc.tile_pool(name="consts", bufs=1))
        php = ctx.enter_context(tc.tile_pool(name="hp", bufs=KNOBS["hp_bufs"]))
        phk = ctx.enter_context(tc.tile_pool(name="hk", bufs=KNOBS["hk_bufs"]))
        pkeys = ctx.enter_context(
            tc.tile_pool(name="keys", bufs=max(1, sum(t_cnts)))
        )
        psmall = ctx.enter_context(tc.tile_pool(name="small", bufs=KNOBS["small_bufs"]))
        pacc = ctx.enter_context(tc.tile_pool(name="acc", bufs=1))
        pq = ctx.enter_context(tc.tile_pool(name="q", bufs=2))
        pps_key = ctx.enter_context(
            tc.tile_pool(name="ps_key", bufs=KNOBS["keyps_bufs"], space="PSUM")
        )
        pps_acc = ctx.enter_context(tc.tile_pool(name="ps_acc", bufs=1, space="PSUM"))
        pps_sm = ctx.enter_context(tc.tile_pool(name="ps_sm", bufs=1, space="PSUM"))

        # ---- constants: one DMA per pack ----
        cf = consts.tile([128, PACKF], F32)
        nc.sync.dma_start(cf, packf)
        cb = consts.tile([128, PACKB], BF16)
        nc.sync.dma_start(cb, packb)

        def wk_sb(c):
            return cb[:, OFFB_WK + c * 512 : OFFB_WK + (c + 1) * 512]

        def wq_sb(c):
            return cb[:, OFFB_WQ + c * 512 : OFFB_WQ + (c + 1) * 512]

        maskb_sb = cf[:, OFF_MASK : OFF_MASK + NT]
        bq_sb = cf[0:1, OFF_BQ : OFF_BQ + H]
        zero_sb = cf[:, OFF_ZERO : OFF_ZERO + 1]
        bk_sb = cb[0:1, OFFB_BK : OFFB_BK + H]
        ones_row_sb = cb[0:1, OFFB_ONESROW : OFFB_ONESROW + 128]
        ones1_sb = cb[:, OFFB_ONES : OFFB_ONES + 1]

        def oh_sb(g):
            return cb[:, OFFB_OH + g * BPC : OFFB_OH + (g + 1) * BPC]

        # Dummy PE ops: observe each const-pack DMA lane once, so no real
        # matmul ever needs two sync-waits (walrus S3_LW limit is one).
        scr = pps_sm.tile([128, H], F32, tag="sm")
        nc.tensor.matmul(scr[0:1, 0:1], ones1_sb, ones1_sb, start=True, stop=True)
        nc.tensor.matmul(scr[0:1, 0:1], zero_sb, zero_sb, start=True, stop=True)

        # per-tile partial sums of h: red_all[j, T*HC + c] = sum_s hT (fp32,
        # written by the prepass TSP accumulators; reduced per group in emit_q)
        red_all = pacc.tile([128, NT * HC], F32)

        keys = [None] * NT
        qrep = [None] * BPC
        numer_ps = pps_acc.tile([BPC, H], F32, tag="numer")
        den_ps = pps_acc.tile([BPC, 1], F32, tag="den")
        nb_total = sum(t_cnts)
        nb_done = 0  # emitted B tiles, to set start/stop on the accumulators

        hp_cur = [None]  # current prepass chunk tile

        def emit_pre_dma(ch):
            h_t = php.tile([128, CHUNK * H], BF16, tag="hp")
            nc.gpsimd.dma_start(h_t, x[ch])
            hp_cur[0] = h_t

        def emit_red(g, t):
            T = g * TPG + t
            hview = hp_cur[0][:, (t % CHUNK) * H : (t % CHUNK + 1) * H]
            for c in range(HC):
                pd = psmall.tile([128, 128], BF16, tag="pd")
                nc.vector.tensor_scalar(
                    pd,
                    hview[:, c * 128 : (c + 1) * 128],
                    1.0,
                    0.0,
                    op0=ALU.mult,
                    op1=ALU.add,
                    accum_out=red_all[:, T * HC + c : T * HC + c + 1],
                )

        def emit_q(g):
            redg = pacc.tile([128, HC], F32, tag=f"rg{g}")
            nc.vector.tensor_reduce(
                redg,
                red_all[
                    :, g * TPG * HC : (g + 1) * TPG * HC
                ].rearrange("p (t c) -> p c t", t=TPG, c=HC),
                axis=mybir.AxisListType.X,
                op=ALU.add,
            )
            maccb = pacc.tile([128, HC], BF16, tag=f"mb{g}")
            nc.vector.tensor_copy(maccb, redg)
            q_ps = pps_sm.tile([128, H], F32, tag="sm")
            for c in range(HC):
                nc.tensor.matmul(
                    q_ps[0:1, :],
                    maccb[:, c : c + 1],
                    wq_sb(c),
                    start=(c == 0),
                    stop=(c == HC - 1),
                )
            q_sb = pq.tile([1, H], F32, tag="q")
            nc.scalar.mul(q_sb, q_ps[0:1, :], 1.0 / S)
            q_b = pq.tile([1, H], BF16, tag="qb")
            nc.vector.tensor_add(q_b, q_sb, bq_sb)
            qrep_ps = pps_sm.tile([128, H], F32, tag="sm")
            nc.tensor.matmul(qrep_ps, ones_row_sb, q_b, start=True, stop=True)
            qrep_g = pq.tile([128, H], BF16, tag="qr")
            nc.vector.tensor_copy(qrep_g, qrep_ps)
            qrep[g] = qrep_g

        hk_cur = [None]

        def emit_key(g, t):
            T = g * TPG + t
            if t % CHUNK == 0:
                h_t = phk.tile([128, CHUNK * H], BF16, tag="hk")
                nc.gpsimd.dma_start(h_t, x[T // CHUNK])
                hk_cur[0] = h_t
            hview = hk_cur[0][:, (t % CHUNK) * H : (t % CHUNK + 1) * H]
            key_ps = pps_key.tile([TOK, H], F32, tag="key")
            nc.tensor.matmul(key_ps, ones_row_sb, bk_sb, start=True, stop=False)
            for c in range(HC):
                nc.tensor.matmul(
                    key_ps,
                    hview[:, c * 128 : (c + 1) * 128],
                    wk_sb(c),
                    start=False,
                    stop=(c == HC - 1),
                )
            key_t = pkeys.tile([TOK, H], BF16, tag="key")
            nc.scalar.activation(key_t, key_ps, AF.Tanh, bias=zero_sb)
            keys[T] = key_t

        def emit_b(g, t):
            nonlocal nb_done
            T = g * TPG + t
            key_t = keys[T]
            sc_t = psmall.tile([TOK, 1], F32, tag="sc")
            if (t * 7) % 10 < KNOBS["stt_dve10"][g]:
                prod = psmall.tile([TOK, H], BF16, tag="prod")
                nc.vector.scalar_tensor_tensor(
                    prod, key_t, 1.0, qrep[g], ALU.mult, ALU.mult, accum_out=sc_t
                )
            else:
                prod = psmall.tile([TOK, H], BF16, tag="prodg")
                nc.gpsimd.tensor_mul(prod, key_t, qrep[g])
                pc = psmall.tile([TOK, H], BF16, tag="pc")
                nc.scalar.activation(pc, prod, AF.Copy, accum_out=sc_t)
            e_b = psmall.tile([TOK, 1], F32, tag="e")
            nc.scalar.activation(e_b, sc_t, AF.Exp, bias=maskb_sb[:, T : T + 1])
            # ei[s, g'] = e[s] * (g' == g): lets one [4, H] PSUM accumulator
            # collect all four groups (matmul outs must start at partition 0)
            ei_t = psmall.tile([TOK, BPC], BF16, tag="ei")
            nc.vector.tensor_scalar_mul(ei_t, oh_sb(g), e_b)
            first = nb_done == 0
            last = nb_done == nb_total - 1
            nc.tensor.matmul(numer_ps, ei_t, key_t, start=first, stop=last)
            nc.tensor.matmul(den_ps, ei_t, ones1_sb, start=first, stop=last)
            nb_done += 1

        # ---- prepass for group 0 ----
        for ch in range(CPG):
            emit_pre_dma(ch)
            for t in range(ch * CHUNK, (ch + 1) * CHUNK):
                emit_red(0, t)
        emit_q(0)

        # ---- windows: keypass(g) || B(g) || prepass(g+1) ----
        stg = KNOBS["stagger"]
        for g in range(BPC):
            m = t_cnts[g]
            # prepass ops for group g+1: [dma, red*CHUNK] per chunk
            pre_ops = []
            if g + 1 < BPC:
                gn = g + 1
                for ch in range(CPG):
                    cg = gn * CPG + ch
                    pre_ops.append(lambda cg=cg: emit_pre_dma(cg))
                    for t in range(ch * CHUNK, (ch + 1) * CHUNK):
                        pre_ops.append(lambda gn=gn, t=t: emit_red(gn, t))
            # spread prepass ops across the m keypass tiles
            npre = len(pre_ops)
            done = 0
            for t in range(m):
                emit_key(g, t)
                want = npre * (t + 1) // m
                while done < want:
                    pre_ops[done]()
                    done += 1
                if t >= stg:
                    emit_b(g, t - stg)
            while done < npre:
                pre_ops[done]()
                done += 1
            for bt in range(max(m - stg, 0), m):
                emit_b(g, bt)
            if g + 1 < BPC:
                emit_q(g + 1)

        # ---- out = numer / den ----
        rcp = pacc.tile([BPC, 1], F32)
        nc.vector.reciprocal(rcp, den_ps)
        out_sb = pacc.tile([BPC, H], F32)
        nc.vector.tensor_scalar_mul(out_sb, numer_ps, rcp)
        nc.sync.dma_start(y, out_sb)


_CACHE = {}


def _get_program(t_cnts=None):
    if t_cnts is None:
        t_cnts = _CACHE.get("last")
        assert t_cnts is not None, "no program built yet"
    t_cnts = tuple(int(t) for t in t_cnts)
    if t_cnts in _CACHE:
        _CACHE["last"] = t_cnts
        return _CACHE[t_cnts]
    nc = bacc.Bacc(None, target_bir_lowering=False, debug=False)
    aps = {
        "x": nc.dram_tensor(
            "x", [NCHUNK, 128, CHUNK * H], BF16, kind="ExternalInput"
        ).ap(),
        "packf": nc.dram_tensor("packf", [128, PACKF], F32, kind="ExternalInput").ap(),
        "packb": nc.dram_tensor("packb", [128, PACKB], BF16, kind="ExternalInput").ap(),
        "y": nc.dram_tensor("y", [BPC, H], F32, kind="ExternalOutput").ap(),
    }
    with tile.TileContext(nc) as tc:
        _build_kernel_body(tc, aps, t_cnts)
    nc.finalize()  # Bacc.compile: wait legalization (EVSEM splits), LDW moves
    _CACHE[t_cnts] = (nc, aps)
    _CACHE["last"] = t_cnts
    return nc, aps


def _plan(lengths):
    """Sort batches by length (desc); rank r -> (core r%8, group r//8).
    Returns (order, t_cnts): order[r] = original batch index; t_cnts[g] =
    tiles of keypass/B work for group g (max over cores, SPMD-shared)."""
    lens = np.asarray(lengths).astype(np.int64)
    order = np.argsort(-lens, kind="stable")
    sl = lens[order].reshape(BPC, NCORES)  # [group, core]
    t_cnts = np.ceil(sl.max(axis=1) / TOK).astype(int)
    return order, tuple(int(t) for t in t_cnts)


def _make_in_maps(hidden_states, Wq, bq, Wk, bk, lengths, order):
    hidden = np.asarray(hidden_states, dtype=np.float32)
    Wq = np.asarray(Wq, dtype=np.float32)
    Wk = np.asarray(Wk, dtype=np.float32)
    bqv = np.asarray(bq, dtype=np.float32)
    bkv = np.asarray(bk, dtype=np.float32)
    lens = np.asarray(lengths).astype(np.int64)

    p = np.arange(128)
    packb = np.zeros((128, PACKB), dtype=BF16NP)
    packb[:, OFFB_WK : OFFB_WK + 2048] = (
        np.ascontiguousarray(Wk.T)
        .reshape(HC, 128, H)
        .transpose(1, 0, 2)
        .reshape(128, 2048)
        .astype(BF16NP)
    )
    packb[:, OFFB_WQ : OFFB_WQ + 2048] = (
        np.ascontiguousarray(Wq.T)
        .reshape(HC, 128, H)
        .transpose(1, 0, 2)
        .reshape(128, 2048)
        .astype(BF16NP)
    )
    packb[0, OFFB_BK : OFFB_BK + H] = bkv.astype(BF16NP)
    packb[0, OFFB_ONESROW : OFFB_ONESROW + 128] = BF16NP(1.0)
    packb[:, OFFB_ONES] = BF16NP(1.0)
    for g in range(BPC):
        packb[:, OFFB_OH + g * BPC + g] = BF16NP(1.0)

    base_packf = np.zeros((128, PACKF), dtype=np.float32)
    base_packf[0, OFF_BQ : OFF_BQ + H] = bqv

    in_maps = []
    t_idx = np.arange(NT)
    for c in range(NCORES):
        bsel = [int(order[g * NCORES + c]) for g in range(BPC)]
        hc = hidden[:, bsel, :]  # [S, 4, 512]
        # -> [chunk, j_local, (tile_in_chunk, cj, s_local)] bf16
        xc = np.ascontiguousarray(
            hc.transpose(1, 0, 2)  # [g, S, H]
            .reshape(BPC, CPG, CHUNK, TOK, HC, 128)  # g, ch, tl, p, cj, j
            .transpose(0, 1, 5, 2, 4, 3)  # g, ch, j, tl, cj, p
        ).reshape(NCHUNK, 128, CHUNK * H).astype(BF16NP)
        packf = base_packf.copy()
        b_of_t = np.array([bsel[g] for g in t_idx // TPG])  # [NT]
        s_full = (t_idx % TPG)[None, :] * TOK + p[:, None]  # [128, NT]
        valid = s_full < lens[b_of_t][None, :]
        packf[:, OFF_MASK : OFF_MASK + NT] = np.where(valid, 0.0, MASK_NEG)
        in_maps.append({"x": xc, "packf": packf, "packb": packb})
    return in_maps


def run(hidden_states, Wq, bq, Wk, bk, lengths, trace=False):
    """Run on 8 cores; returns (output [B, H] fp32, BassKernelResults)."""
    order, t_cnts = _plan(lengths)
    nc, _ = _get_program(t_cnts)
    in_maps = _make_in_maps(hidden_states, Wq, bq, Wk, bk, lengths, order)
    res = run_bass_kernel_spmd(
        nc, in_maps, core_ids=list(range(NCORES)), trace=trace
    )
    rows = np.concatenate([np.asarray(r["y"]) for r in res.results], axis=0)
    out = np.empty((B, H), dtype=np.float32)
    for c in range(NCORES):
        for g in range(BPC):
            out[int(order[g * NCORES + c])] = rows[c * BPC + g]
    return out, res


def kernel(hidden_states, Wq, bq, Wk, bk, lengths):
    out, _ = run(hidden_states, Wq, bq, Wk, bk, lengths)
    return out


# revision 19
# speedup vs baseline: 2.3572x; 1.0002x over previous
"""Trainium2 Bass kernel for nn_Attention_82660940579436.

Computation (see reference):
    q     = mean_s(hidden @ Wq.T + bq)            [B, H]
    key   = tanh(hidden @ Wk.T + bk)              [S, B, H]
    score = einsum('bsh,bh->bs', key, q) + mask   [B, S]
    out   = softmax(score) @ key                  [B, H]

Sharding: data-parallel over batch. B=32 over 8 cores -> 4 batches/core.

v6 design (290us baseline -> 250 -> 224 -> this):
  - Hidden is pre-transposed and bf16-cast ON HOST into the exact SBUF
    layout the key matmuls need ([chunk, j_local, (tile, cj, s)]), DMA'd
    in 8-tile chunks (SWDGE descriptor-gen costs ~1us of Pool time per
    load, so per-tile loads are out).
  - One batch per 128-token tile: 4 groups of 32 tiles, tok = s_local.
  - MASKED-TILE SKIPPING: positions s >= length only reach the output
    through exp(-60) ~ 1e-26, so tiles entirely past a batch's length
    need no key projection / softmax work.  Only q = mean_s(...) needs
    the full sequence.  Batches are SORTED by length on the host and
    assigned to (core, group) slots so each group's max length across
    the 8 cores (the SPMD program is shared) is minimal; the program is
    compiled per set of group tile-counts (cached).  For uniform random
    lengths this skips ~40% of the expensive work.
  - PREPASS/KEYPASS split: a cheap prepass (chunk DMA + per-chunk sums
    via DVE TSP-accumulate + the tiny q chain) runs one group ahead,
    interleaved into the previous group's keypass, so q(g) is ready
    when keypass(g) starts and phase B (softmax) of each tile follows
    its key projection immediately.  Phase A (PE-bound) and phase B
    (DVE/ACT/Pool-bound) overlap everywhere except a short head.
    Chunks are re-loaded for the keypass (DMA device has headroom;
    SBUF does not - keys alone take up to 128KiB/partition).
  - Work splits: bias bk via PE rank-1 matmul; score mul+rowsum fused
    in one DVE scalar_tensor_tensor (TSP-reduce is DVE-only) with a
    per-window fraction diverted to GPSIMD-mul + ACT-accumulate where
    the prepass loads the DVE.

exp() needs no max-subtraction: scores are O(1) by construction, masked
positions get -60 bias -> exp underflows to ~1e-27 (reference's -10000
mask likewise produces exact zeros after its own softmax).

All constants ship in two packed tensors (one fp32, one bf16) loaded by a
single DMA each; two dummy PE ops observe those DMA lanes up front so no
real matmul needs two sync-waits (walrus allows one on a Matmult).
"""

import sys
from contextlib import ExitStack

import numpy as np

if "/opt/trn_rl_repo" not in sys.path:
    sys.path.insert(0, "/opt/trn_rl_repo")

import ml_dtypes  # noqa: E402

import concourse.bacc as bacc  # noqa: E402
import concourse.bass as bass  # noqa: E402
import concourse.mybir as mybir  # noqa: E402
import concourse.tile as tile  # noqa: E402
from concourse.bass_utils import run_bass_kernel_spmd  # noqa: E402

S, B, H = 4096, 32, 512
NCORES = 8
BPC = B // NCORES  # 4 batches per core = 4 groups
NT = 128  # tiles per core
TPG = NT // BPC  # 32 tiles per group
TOK = S // TPG  # 128 tokens (s-positions) per tile
HC = H // 128  # 4 chunks of the H (j / i) dims
CHUNK = 8  # tiles per hidden DMA
NCHUNK = NT // CHUNK
CPG = TPG // CHUNK  # chunks per group
MASK_NEG = -60.0
F32 = mybir.dt.float32
BF16 = mybir.dt.bfloat16
AF = mybir.ActivationFunctionType
ALU = mybir.AluOpType
BF16NP = ml_dtypes.bfloat16

# fp32 const pack layout (offsets in fp32 elements, [128, PACKF] tensor)
OFF_MASK = 0  # [128, NT] mask bias (0 / MASK_NEG), col=global tile
OFF_BQ = 128  # [1, 512] bq row
OFF_ZERO = 640  # [128, 1] zeros (tanh bias)
PACKF = 641
# bf16 const pack layout ([128, PACKB]) — matmul operands live here:
# fp32 matmuls run at 1/4 rate on TRN2, bf16 at full rate.
OFFB_WK = 0  # [128, 2048] WkT chunks
OFFB_WQ = 2048  # [128, 2048] WqT chunks
OFFB_BK = 4096  # [1, 512] bk on partition 0 (PE bias matmul rhs)
OFFB_ONESROW = 4608  # [1, 128] ones on partition 0 (PE bias/bcast lhsT)
OFFB_ONES = 4736  # [128, 1] ones
OFFB_OH = 4738  # [128, 4*4] one-hot rows: cols g*4+g' = (g == g')
PACKB = 4754

# tuning knobs (read at build time)
KNOBS = {
    "hp_bufs": 2,  # prepass chunk buffers
    "hk_bufs": 2,  # keypass chunk buffers
    "keyps_bufs": 5,
    "small_bufs": 6,
    "stagger": 5,  # tiles of keypass(g) emitted before B(g) starts
    # phase B: fraction (in tenths) of tiles whose fused score mul+rowsum
    # runs on DVE (scalar_tensor_tensor), per window; the rest go to
    # GPSIMD-mul + ACT-accumulate.  Window 2 carries the heaviest prepass
    # red load on DVE; window 3 has no prepass.
    "stt_dve10": [8, 6, 4, 9],
}


def _build_kernel_body(tc, aps, t_cnts):
    nc = tc.nc
    x, packf, packb, y = aps["x"], aps["packf"], aps["packb"], aps["y"]

    with ExitStack() as ctx:
        consts = ctx.enter_context(tc.tile_pool(name="consts", bufs=1))
        php = ctx.enter_context(tc.tile_pool(name="hp", bufs=KNOBS["hp_bufs"]))
        phk = ctx.enter_context(tc.tile_pool(name="hk", bufs=KNOBS["hk_bufs"]))
        pkeys = ctx.enter_context(
            tc.tile_pool(name="keys", bufs=max(1, sum(t_cnts)))
        )
        psmall = ctx.enter_context(tc.tile_pool(name="small", bufs=KNOBS["small_bufs"]))
        pacc = ctx.enter_context(tc.tile_pool(name="acc", bufs=1))
        pq = ctx.enter_context(tc.tile_pool(name="q", bufs=2))
        pps_key = ctx.enter_context(
            tc.tile_pool(name="ps_key", bufs=KNOBS["keyps_bufs"], space="PSUM")
        )
        pps_acc = ctx.enter_context(tc.tile_pool(name="ps_acc", bufs=1, space="PSUM"))
        pps_sm = ctx.enter_context(tc.tile_pool(name="ps_sm", bufs=1, space="PSUM"))

        # ---- constants: one DMA per pack ----
        cf = consts.tile([128, PACKF], F32)
        nc.sync.dma_start(cf, packf)
        cb = consts.tile([128, PACKB], BF16)
        nc.sync.dma_start(cb, packb)

        def wk_sb(c):
            return cb[:, OFFB_WK + c * 512 : OFFB_WK + (c + 1) * 512]

        def wq_sb(c):
            return cb[:, OFFB_WQ + c * 512 : OFFB_WQ + (c + 1) * 512]

        maskb_sb = cf[:, OFF_MASK : OFF_MASK + NT]
        bq_sb = cf[0:1, OFF_BQ : OFF_BQ + H]
        zero_sb = cf[:, OFF_ZERO : OFF_ZERO + 1]
        bk_sb = cb[0:1, OFFB_BK : OFFB_BK + H]
        ones_row_sb = cb[0:1, OFFB_ONESROW : OFFB_ONESROW + 128]
        ones1_sb = cb[:, OFFB_ONES : OFFB_ONES + 1]

        def oh_sb(g):
            return cb[:, OFFB_OH + g * BPC : OFFB_OH + (g + 1) * BPC]

        # Dummy PE ops: observe each const-pack DMA lane once, so no real
        # matmul ever needs two sync-waits (walrus S3_LW limit is one).
        scr = pps_sm.tile([128, H], F32, tag="sm")
        nc.tensor.matmul(scr[0:1, 0:1], ones1_sb, ones1_sb, start=True, stop=True)
        nc.tensor.matmul(scr[0:1, 0:1], zero_sb, zero_sb, start=True, stop=True)

        # per-tile partial sums of h: red_all[j, T*HC + c] = sum_s hT (fp32,
        # written by the prepass TSP accumulators; reduced per group in emit_q)
        red_all = pacc.tile([128, NT * HC], F32)

        keys = [None] * NT
        qrep = [None] * BPC
        numer_ps = pps_acc.tile([BPC, H], F32, tag="numer")
        den_ps = pps_acc.tile([BPC, 1], F32, tag="den")
        nb_total = sum(t_cnts)
        nb_done = 0  # emitted B tiles, to set start/stop on the accumulators

        hp_cur = [None]  # current prepass chunk tile

        def emit_pre_dma(ch):
            h_t = php.tile([128, CHUNK * H], BF16, tag="hp")
            nc.gpsimd.dma_start(h_t, x[ch])
            hp_cur[0] = h_t

        def emit_red(g, t):
            T = g * TPG + t
            hview = hp_cur[0][:, (t % CHUNK) * H : (t % CHUNK + 1) * H]
            for c in range(HC):
                pd = psmall.tile([128, 128], BF16, tag="pd")
                nc.vector.tensor_scalar(
                    pd,
                    hview[:, c * 128 : (c + 1) * 128],
                    1.0,
                    0.0,
                    op0=ALU.mult,
                    op1=ALU.add,
                    accum_out=red_all[:, T * HC + c : T * HC + c + 1],
                )

        def emit_q(g):
            redg = pacc.tile([128, HC], F32, tag=f"rg{g}")
            nc.vector.tensor_reduce(
                redg,
                red_all[
                    :, g * TPG * HC : (g + 1) * TPG * HC
                ].rearrange("p (t c) -> p c t", t=TPG, c=HC),
                axis=mybir.AxisListType.X,
                op=ALU.add,
            )
            maccb = pacc.tile([128, HC], BF16, tag=f"mb{g}")
            nc.vector.tensor_copy(maccb, redg)
            q_ps = pps_sm.tile([128, H], F32, tag="sm")
            for c in range(HC):
                nc.tensor.matmul(
                    q_ps[0:1, :],
                    maccb[:, c : c + 1],
                    wq_sb(c),
                    start=(c == 0),
                    stop=(c == HC - 1),
                )
            q_sb = pq.tile([1, H], F32, tag="q")
            nc.scalar.mul(q_sb, q_ps[0:1, :], 1.0 / S)
            q_b = pq.tile([1, H], BF16, tag="qb")
            nc.vector.tensor_add(q_b, q_sb, bq_sb)
            qrep_ps = pps_sm.tile([128, H], F32, tag="sm")
            nc.tensor.matmul(qrep_ps, ones_row_sb, q_b, start=True, stop=True)
            qrep_g = pq.tile([128, H], BF16, tag="qr")
            nc.vector.tensor_copy(qrep_g, qrep_ps)
            qrep[g] = qrep_g

        hk_cur = [None]

        def emit_key(g, t):
            T = g * TPG + t
            if t % CHUNK == 0:
                # only load the tiles this group actually computes
                ntl = min(CHUNK, t_cnts[g] - t)
                h_t = phk.tile([128, CHUNK * H], BF16, tag="hk")
                nc.gpsimd.dma_start(
                    h_t[:, : ntl * H], x[T // CHUNK][:, : ntl * H]
                )
                hk_cur[0] = h_t
            hview = hk_cur[0][:, (t % CHUNK) * H : (t % CHUNK + 1) * H]
            key_ps = pps_key.tile([TOK, H], F32, tag="key")
            nc.tensor.matmul(key_ps, ones_row_sb, bk_sb, start=True, stop=False)
            for c in range(HC):
                nc.tensor.matmul(
                    key_ps,
                    hview[:, c * 128 : (c + 1) * 128],
                    wk_sb(c),
                    start=False,
                    stop=(c == HC - 1),
                )
            key_t = pkeys.tile([TOK, H], BF16, tag="key")
            nc.scalar.activation(key_t, key_ps, AF.Tanh, bias=zero_sb)
            keys[T] = key_t

        def emit_b(g, t):
            nonlocal nb_done
            T = g * TPG + t
            key_t = keys[T]
            sc_t = psmall.tile([TOK, 1], F32, tag="sc")
            if (t * 7) % 10 < KNOBS["stt_dve10"][g]:
                prod = psmall.tile([TOK, H], BF16, tag="prod")
                nc.vector.scalar_tensor_tensor(
                    prod, key_t, 1.0, qrep[g], ALU.mult, ALU.mult, accum_out=sc_t
                )
            else:
                prod = psmall.tile([TOK, H], BF16, tag="prodg")
                nc.gpsimd.tensor_mul(prod, key_t, qrep[g])
                pc = psmall.tile([TOK, H], BF16, tag="pc")
                nc.scalar.activation(pc, prod, AF.Copy, accum_out=sc_t)
            e_b = psmall.tile([TOK, 1], F32, tag="e")
            nc.scalar.activation(e_b, sc_t, AF.Exp, bias=maskb_sb[:, T : T + 1])
            # ei[s, g'] = e[s] * (g' == g): lets one [4, H] PSUM accumulator
            # collect all four groups (matmul outs must start at partition 0)
            ei_t = psmall.tile([TOK, BPC], BF16, tag="ei")
            nc.vector.tensor_scalar_mul(ei_t, oh_sb(g), e_b)
            first = nb_done == 0
            last = nb_done == nb_total - 1
            nc.tensor.matmul(numer_ps, ei_t, key_t, start=first, stop=last)
            nc.tensor.matmul(den_ps, ei_t, ones1_sb, start=first, stop=last)
            nb_done += 1

        # ---- prepass for group 0 ----
        for ch in range(CPG):
            emit_pre_dma(ch)
            for t in range(ch * CHUNK, (ch + 1) * CHUNK):
                emit_red(0, t)
        emit_q(0)

        # ---- windows: keypass(g) || B(g) || prepass(g+1) ----
        stg = KNOBS["stagger"]
        for g in range(BPC):
            m = t_cnts[g]
            # prepass ops for group g+1: [dma, red*CHUNK] per chunk
            pre_ops = []
            if g + 1 < BPC:
                gn = g + 1
                for ch in range(CPG):
                    cg = gn * CPG + ch
                    pre_ops.append(lambda cg=cg: emit_pre_dma(cg))
                    for t in range(ch * CHUNK, (ch + 1) * CHUNK):
                        pre_ops.append(lambda gn=gn, t=t: emit_red(gn, t))
            # spread prepass ops across the m keypass tiles
            npre = len(pre_ops)
            done = 0
            for t in range(m):
                emit_key(g, t)
                want = npre * (t + 1) // m
                while done < want:
                    pre_ops[done]()
                    done += 1
                if t >= stg:
                    emit_b(g, t - stg)
            while done < npre:
                pre_ops[done]()
                done += 1
            for bt in range(max(m - stg, 0), m):
                emit_b(g, bt)
            if g + 1 < BPC:
                emit_q(g + 1)

        # ---- out = numer / den ----
        rcp = pacc.tile([BPC, 1], F32)
        nc.vector.reciprocal(rcp, den_ps)
        out_sb = pacc.tile([BPC, H], F32)
        nc.vector.tensor_scalar_mul(out_sb, numer_ps, rcp)
        nc.sync.dma_start(y, out_sb)


_CACHE = {}


def _get_program(t_cnts=None):
    if t_cnts is None:
        t_cnts = _CACHE.get("last")
        assert t_cnts is not None, "no program built yet"
    t_cnts = tuple(int(t) for t in t_cnts)
    if t_cnts in _CACHE:
        _CACHE["last"] = t_cnts
        return _CACHE[t_cnts]
    nc = bacc.Bacc(None, target_bir_lowering=False, debug=False)
    aps = {
        "x": nc.dram_tensor(
            "x", [NCHUNK, 128, CHUNK * H], BF16, kind="ExternalInput"
        ).ap(),
        "packf": nc.dram_tensor("packf", [128, PACKF], F32, kind="ExternalInput").ap(),
        "packb": nc.dram_tensor("packb", [128, PACKB], BF16, kind="ExternalInput").ap(),
        "y": nc.dram_tensor("y", [BPC, H], F32, kind="ExternalOutput").ap(),
    }
    with tile.TileContext(nc) as tc:
        _build_kernel_body(tc, aps, t_cnts)
    nc.finalize()  # Bacc.compile: wait legalization (EVSEM splits), LDW moves
    _CACHE[t_cnts] = (nc, aps)
    _CACHE["last"] = t_cnts
    return nc, aps


def _plan(lengths):
    """Sort batches by length (desc); rank r -> (core r%8, group r//8).
    Returns (order, t_cnts): order[r] = original batch index; t_cnts[g] =
    tiles of keypass/B work for group g (max over cores, SPMD-shared)."""
    lens = np.asarray(lengths).astype(np.int64)
    order = np.argsort(-lens, kind="stable")
    sl = lens[order].reshape(BPC, NCORES)  # [group, core]
    t_cnts = np.ceil(sl.max(axis=1) / TOK).astype(int)
    return order, tuple(int(t) for t in t_cnts)


def _make_in_maps(hidden_states, Wq, bq, Wk, bk, lengths, order):
    hidden = np.asarray(hidden_states, dtype=np.float32)
    Wq = np.asarray(Wq, dtype=np.float32)
    Wk = np.asarray(Wk, dtype=np.float32)
    bqv = np.asarray(bq, dtype=np.float32)
    bkv = np.asarray(bk, dtype=np.float32)
    lens = np.asarray(lengths).astype(np.int64)

    p = np.arange(128)
    packb = np.zeros((128, PACKB), dtype=BF16NP)
    packb[:, OFFB_WK : OFFB_WK + 2048] = (
        np.ascontiguousarray(Wk.T)
        .reshape(HC, 128, H)
        .transpose(1, 0, 2)
        .reshape(128, 2048)
        .astype(BF16NP)
    )
    packb[:, OFFB_WQ : OFFB_WQ + 2048] = (
        np.ascontiguousarray(Wq.T)
        .reshape(HC, 128, H)
        .transpose(1, 0, 2)
        .reshape(128, 2048)
        .astype(BF16NP)
    )
    packb[0, OFFB_BK : OFFB_BK + H] = bkv.astype(BF16NP)
    packb[0, OFFB_ONESROW : OFFB_ONESROW + 128] = BF16NP(1.0)
    packb[:, OFFB_ONES] = BF16NP(1.0)
    for g in range(BPC):
        packb[:, OFFB_OH + g * BPC + g] = BF16NP(1.0)

    base_packf = np.zeros((128, PACKF), dtype=np.float32)
    base_packf[0, OFF_BQ : OFF_BQ + H] = bqv

    in_maps = []
    t_idx = np.arange(NT)
    for c in range(NCORES):
        bsel = [int(order[g * NCORES + c]) for g in range(BPC)]
        hc = hidden[:, bsel, :]  # [S, 4, 512]
        # -> [chunk, j_local, (tile_in_chunk, cj, s_local)] bf16
        xc = np.ascontiguousarray(
            hc.transpose(1, 0, 2)  # [g, S, H]
            .reshape(BPC, CPG, CHUNK, TOK, HC, 128)  # g, ch, tl, p, cj, j
            .transpose(0, 1, 5, 2, 4, 3)  # g, ch, j, tl, cj, p
        ).reshape(NCHUNK, 128, CHUNK * H).astype(BF16NP)
        packf = base_packf.copy()
        b_of_t = np.array([bsel[g] for g in t_idx // TPG])  # [NT]
        s_full = (t_idx % TPG)[None, :] * TOK + p[:, None]  # [128, NT]
        valid = s_full < lens[b_of_t][None, :]
        packf[:, OFF_MASK : OFF_MASK + NT] = np.where(valid, 0.0, MASK_NEG)
        in_maps.append({"x": xc, "packf": packf, "packb": packb})
    return in_maps


def run(hidden_states, Wq, bq, Wk, bk, lengths, trace=False):
    """Run on 8 cores; returns (output [B, H] fp32, BassKernelResults)."""
    order, t_cnts = _plan(lengths)
    nc, _ = _get_program(t_cnts)
    in_maps = _make_in_maps(hidden_states, Wq, bq, Wk, bk, lengths, order)
    res = run_bass_kernel_spmd(
        nc, in_maps, core_ids=list(range(NCORES)), trace=trace
    )
    rows = np.concatenate([np.asarray(r["y"]) for r in res.results], axis=0)
    out = np.empty((B, H), dtype=np.float32)
    for c in range(NCORES):
        for g in range(BPC):
            out[int(order[g * NCORES + c])] = rows[c * BPC + g]
    return out, res


def kernel(hidden_states, Wq, bq, Wk, bk, lengths):
    out, _ = run(hidden_states, Wq, bq, Wk, bk, lengths)
    return out
